# revision 1
# baseline (speedup 1.0000x reference)
"""DLASSO-GNN-Hypernet ADMM forward on 8 Trainium2 NeuronCores (Bass/Tile).

Sharding: data-parallel over batch (8 samples/core). Graph message passing is
done as dense per-sample [50,50] operators built host-side from the integer
edge lists (count matrices + symmetric normalization); all float compute runs
on-device in fp32. The 52MB Wd1 matrix is streamed from HBM every ADMM step
(ring-buffered); everything else is SBUF-resident.

Layouts per core (8 samples, 2-sample "packs", pack rows r = b_loc*50 + p):
  transposed activations: [feat, col] with col = p*8 + b   (feature on SBUF
  partitions; per-node column groups of 8 contiguous for per-node matmuls)
  natural ADMM states:    4 tiles [100, 200] (pack-stacked rows)
"""
import numpy as np

import concourse.bass as bass
import concourse.bacc as bacc
import concourse.tile as tile
import concourse.mybir as mybir
from concourse import bass_utils

F32 = mybir.dt.float32
AF = mybir.ActivationFunctionType
ALU = mybir.AluOpType

B, P, M, N, H, K = 64, 50, 100, 200, 128, 5
BC = 8                      # samples per core
NPACK, PKW = 4, 100         # packs of 2 samples -> 100 rows each
NC_CORES = 8
FOUT = [128, 256, 512, 512, 512]        # GCN layer output dims
FIN = [400, 128, 256, 512, 512]
LN_EPS = 1e-5
RING_BUFS = 5

KT400 = [(0, 128), (128, 72), (200, 128), (328, 72)]
KT200 = [(0, 128), (128, 72)]


def _kt(dim):
    if dim == 400:
        return KT400
    return [(s, min(128, dim - s)) for s in range(0, dim, 128)]


def _pk(ap_2d, pack):
    """[*, 400]-col AP (col = b*50+p) -> pack's contiguous 100 cols."""
    return ap_2d[:, PKW * pack:PKW * pack + PKW]


def _nd(ap_2d, p):
    """[*, 400]-col AP (col = b*50+p) -> node p's 8 sample-cols (stride 50)."""
    return ap_2d.rearrange("a (b p) -> a p b", p=P)[:, p, :]


def _newton_rsqrt(nc, pool, t_ap, shape, tag):
    """rstd = 1/sqrt(t) to ~1 ulp. ACT sqrt seed (loose LUT) + 2 Newton steps.
    r <- r*(1.5 - 0.5*t*r^2)."""
    s0 = pool.tile(shape, F32, tag=tag + "s0", name=tag + "s0")
    nc.scalar.activation(s0[:], t_ap, AF.Sqrt)
    r = pool.tile(shape, F32, tag=tag + "r", name=tag + "r")
    nc.vector.reciprocal(r[:], s0[:])
    tmp = pool.tile(shape, F32, tag=tag + "t", name=tag + "t")
    for _ in range(2):
        nc.vector.tensor_mul(tmp[:], r[:], r[:])
        nc.vector.tensor_mul(tmp[:], tmp[:], t_ap)
        nc.vector.tensor_scalar(tmp[:], tmp[:], -0.5, 1.5, ALU.mult, ALU.add)
        nc.vector.tensor_mul(r[:], r[:], tmp[:])
    return r


def build_nc(skip_bv=True, skip_ld=True, skip_bd=True):
    """skip_* : omit device work for bias/affine params that are all-zero /
    identity in the given inputs (decided host-side)."""
    nc = bacc.Bacc("TRN2", target_bir_lowering=False, debug=False,
                   enable_asserts=False, num_devices=NC_CORES)

    def din(name, shape):
        return nc.dram_tensor(name, list(shape), F32, kind="ExternalInput").ap()

    d_A0 = din("A0l", (M, P * N))
    d_bT = din("bT", (M, BC * P))
    d_y0T = din("y0T", (N, BC * P))
    d_y0n = din("y0n", (NPACK * PKW, N))
    d_U0n = din("U0n", (NPACK * PKW, N))
    d_d0n = din("d0n", (NPACK * PKW, N))
    d_Gt = din("GhatT", (NPACK * PKW, PKW))
    d_Gte = din("GhatE", (NPACK * 2, PKW))
    d_Lt = din("LdT", (NPACK * PKW, PKW))
    d_sel = din("sel", (BC, NPACK * PKW))
    d_snb = din("snb", (PKW, NPACK))
    d_id = din("ident", (128, 128))
    d_W = [din(f"W{l+1}f", (FIN[l], FOUT[l])) for l in range(5)]
    d_bv = [din(f"bv{l+1}", (2, FOUT[l])) for l in range(5)]
    d_Wd1 = din("Wd1", (P * 512, 512))
    d_Wd2 = din("Wd2", (512, 256))
    d_Wd3 = din("Wd3", (256, 128))
    d_Wfc = din("Wfc", (128, 4))
    d_bdr = [din(n, (1, f)) for n, f in
             (("bd1r", 512), ("bd2r", 256), ("bd3r", 128), ("bfcr", 4))]
    d_ldg = [din(n, (BC, f)) for n, f in
             (("ld1gb", 512), ("ld2gb", 256), ("ld3gb", 128))]
    d_ldb = [din(n, (BC, f)) for n, f in
             (("ld1bb", 512), ("ld2bb", 256), ("ld3bb", 128))]
    d_g5 = din("gs5c", (128, 4))
    d_b5 = din("be5c", (128, 4))
    d_lng = din("lngc", (128, 4))
    d_lnb = din("lnbc", (128, 4))
    d_out = nc.dram_tensor("Ys", [K, NPACK * PKW, N], F32,
                           kind="ExternalOutput").ap()

    with tile.TileContext(nc) as tc:
        from contextlib import ExitStack
        es = ExitStack()
        with es:
            cp = es.enter_context(tc.tile_pool(name="consts", bufs=1))

            def load(dram_ap, shape, tag):
                t = cp.tile(list(shape), F32, tag=tag, name=tag)
                nc.sync.dma_start(out=t[:], in_=dram_ap)
                return t

            ident = load(d_id, (128, 128), "ident")
            Gt = [load(d_Gt[PKW * m:PKW * m + PKW, :], (PKW, PKW), f"Gt{m}")
                  for m in range(NPACK)]
            Gte = [load(d_Gte[2 * m:2 * m + 2, :], (2, PKW), f"Gte{m}")
                   for m in range(NPACK)]
            Lt = [load(d_Lt[PKW * m:PKW * m + PKW, :], (PKW, PKW), f"Lt{m}")
                  for m in range(NPACK)]
            sel = load(d_sel, (BC, NPACK * PKW), "sel")
            snb = load(d_snb, (PKW, NPACK), "snb")
            Wt = {}
            for l in range(5):
                for ki, (ks, kz) in enumerate(_kt(FIN[l])):
                    Wt[(l, ki)] = load(d_W[l][ks:ks + kz, :], (kz, FOUT[l]),
                                       f"W{l}_{ki}")
            bv = (None if skip_bv else
                  [load(d_bv[l], (2, FOUT[l]), f"bv{l}") for l in range(5)])
            Wd2t = [load(d_Wd2[s:s + z, :], (z, 256), f"wd2_{s}")
                    for s, z in _kt(512)]
            Wd3t = [load(d_Wd3[s:s + z, :], (z, 128), f"wd3_{s}")
                    for s, z in _kt(256)]
            Wfct = load(d_Wfc, (128, 4), "wfc")
            if skip_bd:
                bdr = [None, None, None, load(d_bdr[3], (1, 4), "bdr3")]
            else:
                bdr = [load(d_bdr[i], d_bdr[i].shape, f"bdr{i}")
                       for i in range(4)]
            ldg = ldb = None
            if not skip_ld:
                ldg = [load(d_ldg[i], d_ldg[i].shape, f"ldg{i}")
                       for i in range(3)]
                ldb = [load(d_ldb[i], d_ldb[i].shape, f"ldb{i}")
                       for i in range(3)]
            g5c = load(d_g5, (128, 4), "g5c")
            b5c = load(d_b5, (128, 4), "b5c")
            lngc = load(d_lng, (128, 4), "lngc")
            lnbc = load(d_lnb, (128, 4), "lnbc")

            ones_col = cp.tile([128, 1], F32, tag="ones_col")
            nc.vector.memset(ones_col[:], 1.0)
            ones_row = cp.tile([1, 128], F32, tag="ones_row")
            nc.vector.memset(ones_row[:], 1.0)

            yT = [cp.tile([128, BC * P], F32, tag="yT0", name="yT0"),
                  cp.tile([72, BC * P], F32, tag="yT1", name="yT1")]
            nc.sync.dma_start(out=yT[0][:], in_=d_y0T[0:128, :])
            nc.sync.dma_start(out=yT[1][:], in_=d_y0T[128:200, :])
            y_nat, U_nat, dl_nat, Atb_nat = [
                [cp.tile([PKW, N], F32, tag=f"{nm}{m}", name=f"{nm}{m}")
                 for m in range(NPACK)]
                for nm in ("y", "U", "dl", "Atb")]
            for m in range(NPACK):
                for tl, src in ((y_nat, d_y0n), (U_nat, d_U0n), (dl_nat, d_d0n)):
                    nc.sync.dma_start(out=tl[m][:],
                                      in_=src[PKW * m:PKW * m + PKW, :])
            xnt = [cp.tile([z, BC * P], F32, tag=f"xnt{i}", name=f"xnt{i}")
                   for i, (s, z) in enumerate(KT400)]
            hT = [cp.tile([128, BC * P], F32, tag=f"h{i}", name=f"h{i}")
                  for i in range(4)]          # also holds enc^T after LN
            AtA = [cp.tile([128, P * N], F32, tag="ata0", name="ata0"),
                   cp.tile([72, P * N], F32, tag="ata1", name="ata1")]

            # ---------------- setup: AtA = A0^T A0, Atb = A0^T b ----------
            with tc.tile_pool(name="setup", bufs=1) as sp, \
                 tc.tile_pool(name="ps_setup", bufs=2, space="PSUM") as pps:
                bT = sp.tile([M, BC * P], F32, tag="bTs")
                nc.sync.dma_start(out=bT[:], in_=d_bT)
                HP = P * N // 2
                for half in range(2):
                    A0 = sp.tile([M, HP], F32, tag="A0", name="A0")
                    nc.sync.dma_start(out=A0[:],
                                      in_=d_A0[:, half * HP:(half + 1) * HP])
                    for p in range(half * (P // 2), (half + 1) * (P // 2)):
                        a0p = A0[:, N * p - half * HP:N * p - half * HP + N]
                        for i, (s, z) in enumerate(KT200):
                            ps = pps.tile([z, N], F32, tag="sat", name="sat")
                            nc.tensor.matmul(ps[:], lhsT=a0p[:, s:s + z],
                                             rhs=a0p, start=True, stop=True)
                            nc.vector.tensor_copy(AtA[i][:, N * p:N * p + N],
                                                  ps[:])
                            pb = pps.tile([z, BC], F32, tag="sab", name="sab")
                            nc.tensor.matmul(pb[:], lhsT=a0p[:, s:s + z],
                                             rhs=_nd(bT[:], p),
                                             start=True, stop=True)
                            nc.vector.tensor_copy(
                                _nd(xnt[2 + i][:], p), pb[:])
                for m in range(NPACK):
                    for i, (s, z) in enumerate(KT200):
                        pt = pps.tile([PKW, 128], F32, tag="satr", name="satr")
                        nc.tensor.transpose(pt[:, :z], _pk(xnt[2 + i][:], m),
                                            ident[:z, :z])
                        nc.vector.tensor_copy(Atb_nat[m][:, s:s + z],
                                              pt[:, :z])

            ring = es.enter_context(tc.tile_pool(name="ring", bufs=RING_BUFS))

            # ---------------- K ADMM steps --------------------------------
            for k in range(K):
                sw = tc.alloc_tile_pool(name=f"sw{k}", bufs=1)
                # -- AtAy^T into xnt[0], xnt[1] --
                with tc.tile_pool(name=f"ps_aty{k}", bufs=2,
                                  space="PSUM") as pp:
                    for p in range(P):
                        for i, (s, z) in enumerate(KT200):
                            ps = pp.tile([z, BC], F32, tag=f"aty{i}",
                                         name="aty")
                            for j, (js, jz) in enumerate(KT200):
                                nc.tensor.matmul(
                                    ps[:],
                                    lhsT=AtA[j][:, N * p + s:N * p + s + z],
                                    rhs=_nd(yT[j][:], p),
                                    start=(j == 0), stop=(j == 1))
                            nc.vector.tensor_copy(
                                _nd(xnt[i][:], p), ps[:])
                # -- g = AtAy - Atb (natural, via PE transpose) --
                g_t = [sw.tile([PKW, N], F32, tag="g", name=f"g{m}", bufs=4)
                       for m in range(NPACK)]
                with tc.tile_pool(name=f"ps_atr{k}", bufs=2,
                                  space="PSUM") as pp:
                    for m in range(NPACK):
                        for i, (s, z) in enumerate(KT200):
                            pt = pp.tile([PKW, 128], F32, tag="atr",
                                         name="atr")
                            nc.tensor.transpose(pt[:, :z], _pk(xnt[i][:], m),
                                                ident[:z, :z])
                            nc.vector.tensor_sub(g_t[m][:, s:s + z],
                                                 pt[:, :z],
                                                 Atb_nat[m][:, s:s + z])

                # -- GCN: 5 layers, transposed chain --
                cur = xnt
                with tc.tile_pool(name=f"ps_gcn{k}", bufs=2,
                                  space="PSUM") as pp, \
                     tc.tile_pool(name=f"gcnw{k}", bufs=2) as gw:
                    for l in range(5):
                        fo = FOUT[l]
                        nxt = (hT if l == 4 else
                               [gw.tile([128, BC * P], F32, tag=f"xt{i}",
                                        name=f"xt{i}")
                                for i in range((fo + 127) // 128)])
                        for m in range(NPACK):
                            psF = pp.tile([PKW, 512], F32, tag="F", name="F")
                            kts = _kt(FIN[l])
                            for ki, (ks, kz) in enumerate(kts):
                                nc.tensor.matmul(
                                    psF[:, :fo], lhsT=_pk(cur[ki][:], m),
                                    rhs=Wt[(l, ki)][:],
                                    start=(ki == 0), stop=(ki == len(kts) - 1))
                            Fsb = gw.tile([PKW, 512], F32, tag="Fsb",
                                          name="Fsb")
                            nc.scalar.copy(Fsb[:, :fo], psF[:, :fo])
                            for mi, (ms, mz) in enumerate(_kt(fo)):
                                psZ = pp.tile([128, PKW], F32, tag="Z",
                                              name="Z")
                                nc.tensor.matmul(
                                    psZ[:mz, :], lhsT=Fsb[:, ms:ms + mz],
                                    rhs=Gt[m][:], start=True, stop=skip_bv)
                                if not skip_bv:
                                    nc.tensor.matmul(
                                        psZ[:mz, :],
                                        lhsT=bv[l][:, ms:ms + mz],
                                        rhs=Gte[m][:], start=False, stop=True)
                                zt = gw.tile([128, PKW], F32, tag="ztmp",
                                             name="ztmp")
                                nc.vector.tensor_copy(zt[:mz, :], psZ[:mz, :])
                                nc.vector.scalar_tensor_tensor(
                                    _pk(nxt[mi][:mz, :], m), zt[:mz, :], 0.01,
                                    zt[:mz, :], ALU.mult, ALU.max)
                        cur = nxt

                # -- bn5 + LayerNorm over features (transposed) --
                with tc.tile_pool(name=f"ps_ln{k}", bufs=2, space="PSUM") as pp, \
                     tc.tile_pool(name=f"lnw{k}", bufs=1) as lw:
                    for i in range(4):
                        nc.vector.tensor_scalar(
                            hT[i][:], hT[i][:], g5c[:, i:i + 1],
                            b5c[:, i:i + 1], ALU.mult, ALU.add)
                    psmu = pp.tile([1, BC * P], F32, tag="mu", name="psmu")
                    for i in range(4):
                        nc.tensor.matmul(psmu[:], lhsT=ones_col[:],
                                         rhs=hT[i][:], start=(i == 0),
                                         stop=(i == 3))
                    mu_r = lw.tile([1, BC * P], F32, tag="mu_r")
                    nc.scalar.mul(mu_r[:], psmu[:], 1.0 / 512.0)
                    pssq = pp.tile([1, BC * P], F32, tag="sq", name="pssq")
                    for i in range(4):
                        hsq = lw.tile([128, BC * P], F32, tag="hsq",
                                      name="hsq", bufs=2)
                        nc.vector.tensor_mul(hsq[:], hT[i][:], hT[i][:])
                        nc.tensor.matmul(pssq[:], lhsT=ones_col[:], rhs=hsq[:],
                                         start=(i == 0), stop=(i == 3))
                    var_r = lw.tile([1, BC * P], F32, tag="var_r")
                    nc.scalar.mul(var_r[:], pssq[:], 1.0 / 512.0)
                    t_r = lw.tile([1, BC * P], F32, tag="t_r")
                    nc.vector.tensor_mul(t_r[:], mu_r[:], mu_r[:])
                    nc.vector.tensor_sub(var_r[:], var_r[:], t_r[:])
                    nc.vector.tensor_scalar_add(var_r[:], var_r[:], LN_EPS)
                    rs_r = _newton_rsqrt(nc, lw, var_r[:], [1, BC * P], "lnr")
                    bco = {}
                    for nm, row in (("mu", mu_r), ("rs", rs_r)):
                        psb = pp.tile([128, BC * P], F32, tag="bc" + nm,
                                      name="psb")
                        nc.tensor.matmul(psb[:], lhsT=ones_row[:], rhs=row[:],
                                         start=True, stop=True)
                        sb = lw.tile([128, BC * P], F32, tag="bcs" + nm,
                                     name="bcs")
                        nc.vector.tensor_copy(sb[:], psb[:])
                        bco[nm] = sb
                    for i in range(4):
                        nc.vector.tensor_sub(hT[i][:], hT[i][:], bco["mu"][:])
                        nc.vector.tensor_mul(hT[i][:], hT[i][:], bco["rs"][:])
                        nc.vector.tensor_scalar(
                            hT[i][:], hT[i][:], lngc[:, i:i + 1],
                            lnbc[:, i:i + 1], ALU.mult, ALU.add)

                # -- dense hyp-net (Wd1 streamed from HBM) --
                scal, nal = [], []
                with tc.tile_pool(name=f"ps_d{k}", bufs=1, space="PSUM") as pz, \
                     tc.tile_pool(name=f"ps_dt{k}", bufs=2, space="PSUM") as pt, \
                     tc.tile_pool(name=f"dw{k}", bufs=1) as dw:
                    psz1 = pz.tile([BC, 512], F32, tag="z1", name="psz1")
                    for t in range(P):
                        rg = ring.tile([128, 4 * 512], F32, tag="wd1",
                                       name="wd1")
                        src = d_Wd1[512 * t:512 * t + 512, :].rearrange(
                            "(c r) n -> r c n", r=128)
                        nc.sync.dma_start(
                            out=rg[:].rearrange("r (c n) -> r c n", c=4),
                            in_=src)
                        for c in range(4):
                            nc.tensor.matmul(
                                psz1[:], lhsT=_nd(hT[c][:], t),
                                rhs=rg[:, 512 * c:512 * c + 512],
                                start=(t == 0 and c == 0),
                                stop=(t == P - 1 and c == 3 and skip_bd))
                    if not skip_bd:
                        nc.tensor.matmul(psz1[:], lhsT=ones_row[:, :BC],
                                         rhs=bdr[0][:], start=False, stop=True)

                    def dense_ln_lrelu(ps_ap, fdim, li):
                        zr = dw.tile([BC, 512], F32, tag="d_zr", name="zr")
                        nc.vector.tensor_copy(zr[:, :fdim], ps_ap)
                        mu = dw.tile([BC, 1], F32, tag="d_mu", name="mu")
                        nc.vector.tensor_reduce(mu[:], zr[:, :fdim],
                                                mybir.AxisListType.X, ALU.add)
                        nc.vector.tensor_scalar_mul(mu[:], mu[:], 1.0 / fdim)
                        sq = dw.tile([BC, 512], F32, tag="d_sq", name="sq")
                        nc.vector.tensor_mul(sq[:, :fdim], zr[:, :fdim],
                                             zr[:, :fdim])
                        vr = dw.tile([BC, 1], F32, tag="d_vr", name="vr")
                        nc.vector.tensor_reduce(vr[:], sq[:, :fdim],
                                                mybir.AxisListType.X, ALU.add)
                        nc.vector.tensor_scalar_mul(vr[:], vr[:], 1.0 / fdim)
                        mm = dw.tile([BC, 1], F32, tag="d_mm", name="mm")
                        nc.vector.tensor_mul(mm[:], mu[:], mu[:])
                        nc.vector.tensor_sub(vr[:], vr[:], mm[:])
                        nc.vector.tensor_scalar_add(vr[:], vr[:], LN_EPS)
                        rs = _newton_rsqrt(nc, dw, vr[:], [BC, 1], "dn")
                        z = dw.tile([BC, 512], F32, tag="d_z", name="z")
                        nc.vector.tensor_scalar(z[:, :fdim], zr[:, :fdim],
                                                mu[:], rs[:],
                                                ALU.subtract, ALU.mult)
                        if not skip_ld:
                            nc.vector.tensor_mul(z[:, :fdim], z[:, :fdim],
                                                 ldg[li][:, :fdim])
                            nc.vector.tensor_add(z[:, :fdim], z[:, :fdim],
                                                 ldb[li][:, :fdim])
                        nc.vector.scalar_tensor_tensor(
                            z[:, :fdim], z[:, :fdim], 0.01, z[:, :fdim],
                            ALU.mult, ALU.max)
                        return z

                    def transpose_cols(z_sb, fdim):
                        outs = []
                        for ci, (cs, cz) in enumerate(_kt(fdim)):
                            pty = pt.tile([128, BC], F32, tag="zt", name="pty")
                            nc.tensor.transpose(pty[:cz, :],
                                                z_sb[:, cs:cs + cz],
                                                ident[:BC, :BC])
                            zz = dw.tile([128, BC], F32, tag="d_zt", bufs=4,
                                         name=f"zz{ci}")
                            nc.vector.tensor_copy(zz[:cz, :], pty[:cz, :])
                            outs.append((zz, cz))
                        return outs

                    z1 = dense_ln_lrelu(psz1[:], 512, 0)
                    z1t = transpose_cols(z1, 512)
                    psz2 = pz.tile([BC, 256], F32, tag="z2", name="psz2")
                    for ci, (zz, cz) in enumerate(z1t):
                        nc.tensor.matmul(psz2[:], lhsT=zz[:cz, :],
                                         rhs=Wd2t[ci][:], start=(ci == 0),
                                         stop=(ci == 3 and skip_bd))
                    if not skip_bd:
                        nc.tensor.matmul(psz2[:], lhsT=ones_row[:, :BC],
                                         rhs=bdr[1][:], start=False, stop=True)
                    z2 = dense_ln_lrelu(psz2[:], 256, 1)
                    z2t = transpose_cols(z2, 256)
                    psz3 = pz.tile([BC, 128], F32, tag="z3", name="psz3")
                    for ci, (zz, cz) in enumerate(z2t):
                        nc.tensor.matmul(psz3[:], lhsT=zz[:cz, :],
                                         rhs=Wd3t[ci][:], start=(ci == 0),
                                         stop=(ci == 1 and skip_bd))
                    if not skip_bd:
                        nc.tensor.matmul(psz3[:], lhsT=ones_row[:, :BC],
                                         rhs=bdr[2][:], start=False, stop=True)
                    z3 = dense_ln_lrelu(psz3[:], 128, 2)
                    z3t = transpose_cols(z3, 128)
                    psfc = pz.tile([BC, 4], F32, tag="fc", name="psfc")
                    nc.tensor.matmul(psfc[:], lhsT=z3t[0][0][:128, :],
                                     rhs=Wfct[:], start=True, stop=False)
                    nc.tensor.matmul(psfc[:], lhsT=ones_row[:, :BC],
                                     rhs=bdr[3][:], start=False, stop=True)
                    hyp = dw.tile([BC, 4], F32, tag="hyp", name="hyp")
                    nc.scalar.activation(hyp[:], psfc[:], AF.Sigmoid)
                    nc.vector.tensor_scalar(hyp[:], hyp[:], 0.9999, 1e-4,
                                            ALU.min, ALU.max)
                    for m in range(NPACK):
                        pss = pt.tile([PKW, 4], F32, tag="scal", name="pss")
                        nc.tensor.matmul(pss[:],
                                         lhsT=sel[:, PKW * m:PKW * m + PKW],
                                         rhs=hyp[:], start=True, stop=True)
                        sc = sw.tile([PKW, 4], F32, tag="sc", bufs=4,
                                     name=f"sc{m}")
                        nc.vector.tensor_copy(sc[:], pss[:])
                        na = sw.tile([PKW, 1], F32, tag="na", bufs=4,
                                     name=f"na{m}")
                        nc.vector.tensor_scalar_mul(na[:], sc[:, 0:1], -1.0)
                        scal.append(sc)
                        nal.append(na)

                # -- ADMM update (natural layout) --
                with tc.tile_pool(name=f"ps_ad{k}", bufs=2, space="PSUM") as pp, \
                     tc.tile_pool(name=f"adw{k}", bufs=2) as aw:
                    for m in range(NPACK):
                        g = g_t[m]
                        sg = aw.tile([PKW, N], F32, tag="sg", name="sg")
                        nc.scalar.activation(sg[:], y_nat[m][:], AF.Sign)
                        nc.vector.scalar_tensor_tensor(
                            g[:], sg[:], scal[m][:, 1:2], g[:],
                            ALU.mult, ALU.add)
                        nc.vector.scalar_tensor_tensor(
                            g[:], U_nat[m][:], snb[:, m:m + 1], g[:],
                            ALU.mult, ALU.add)
                        nc.vector.scalar_tensor_tensor(
                            g[:], dl_nat[m][:], scal[m][:, 2:3], g[:],
                            ALU.mult, ALU.add)
                        nc.vector.tensor_scalar(g[:], g[:], 10.0, -10.0,
                                                ALU.min, ALU.max)
                        nc.vector.scalar_tensor_tensor(
                            y_nat[m][:], g[:], nal[m][:], y_nat[m][:],
                            ALU.mult, ALU.add)
                        nc.vector.tensor_scalar(y_nat[m][:], y_nat[m][:],
                                                100.0, -100.0,
                                                ALU.min, ALU.max)
                        nc.sync.dma_start(
                            out=d_out[k, PKW * m:PKW * m + PKW, :],
                            in_=y_nat[m][:])
                        psd = pp.tile([PKW, N], F32, tag="dl", name="psd")
                        nc.tensor.matmul(psd[:], lhsT=Lt[m][:],
                                         rhs=y_nat[m][:], start=True,
                                         stop=True)
                        nc.vector.tensor_scalar(dl_nat[m][:], psd[:],
                                                20.0, -20.0, ALU.min, ALU.max)
                        nc.vector.scalar_tensor_tensor(
                            U_nat[m][:], dl_nat[m][:], scal[m][:, 3:4],
                            U_nat[m][:], ALU.mult, ALU.add)
                        nc.vector.tensor_scalar(U_nat[m][:], U_nat[m][:],
                                                100.0, -100.0,
                                                ALU.min, ALU.max)
                        if k < K - 1:
                            for j, (js, jz) in enumerate(KT200):
                                pty = pp.tile([128, PKW], F32, tag="ytr",
                                              name="pty2")
                                nc.tensor.transpose(
                                    pty[:jz, :], y_nat[m][:, js:js + jz],
                                    ident[:PKW, :PKW])
                                nc.vector.tensor_copy(
                                    _pk(yT[j][:], m), pty[:jz, :])
                sw.release()
    nc.compile()
    return nc


_NC_CACHE = {}


def _host_inputs(inputs):
    """Build all per-core DRAM arrays (numpy, fp32). Returns (in_maps, flags)."""
    f32 = np.float32
    BN_SCALE = f32(1.0) / np.sqrt(f32(1.0) + f32(1e-5))
    b_in = np.ascontiguousarray(np.asarray(inputs['b'], f32)[..., 0])
    A0 = np.ascontiguousarray(np.asarray(inputs['A'], f32)[0])
    edge = np.asarray(inputs['edge_index'])
    y0 = np.ascontiguousarray(np.asarray(inputs['y0'], f32)[..., 0])
    U0 = np.ascontiguousarray(np.asarray(inputs['U0'], f32)[..., 0])
    d0 = np.ascontiguousarray(np.asarray(inputs['delta0'], f32)[..., 0])

    Ws = [np.asarray(inputs['W%d' % i], f32) for i in range(1, 6)]
    bs = [np.asarray(inputs['b%d' % i], f32) for i in range(1, 6)]
    gs = [np.asarray(inputs['g%d' % i], f32) * BN_SCALE for i in range(1, 6)]
    bes = [np.asarray(inputs['be%d' % i], f32) for i in range(1, 6)]
    Wf = [Ws[0]] + [gs[l - 1][:, None] * Ws[l] for l in range(1, 5)]
    vs = [np.zeros(FOUT[0], f32)] + [(bes[l - 1] @ Ws[l]).astype(f32)
                                     for l in range(1, 5)]
    lds = [(np.asarray(inputs['ld%dg' % i], f32),
            np.asarray(inputs['ld%db' % i], f32)) for i in (1, 2, 3)]
    bds = [np.asarray(inputs['bd%d' % i], f32) for i in (1, 2, 3)]

    flags = dict(
        skip_bv=bool(all(np.all(bs[l] == 0) and np.all(vs[l] == 0)
                         for l in range(5))),
        skip_ld=bool(all(np.all(g == 1) and np.all(bb == 0)
                         for g, bb in lds)),
        skip_bd=bool(all(np.all(bd == 0) for bd in bds)),
    )

    shared = {'A0l': A0.transpose(1, 0, 2).reshape(M, P * N).copy(),
              'ident': np.eye(128, dtype=f32),
              'Wd1': np.asarray(inputs['Wd1'], f32),
              'Wd2': np.asarray(inputs['Wd2'], f32),
              'Wd3': np.asarray(inputs['Wd3'], f32),
              'Wfc': np.asarray(inputs['Wfc'], f32),
              'bd1r': bds[0][None, :], 'bd2r': bds[1][None, :],
              'bd3r': bds[2][None, :],
              'bfcr': np.asarray(inputs['bfc'], f32)[None, :],
              'gs5c': np.ascontiguousarray(gs[4].reshape(4, 128).T),
              'be5c': np.ascontiguousarray(bes[4].reshape(4, 128).T),
              'lngc': np.ascontiguousarray(
                  np.asarray(inputs['ln_g'], f32).reshape(4, 128).T),
              'lnbc': np.ascontiguousarray(
                  np.asarray(inputs['ln_b'], f32).reshape(4, 128).T)}
    for l in range(5):
        shared[f'W{l+1}f'] = np.ascontiguousarray(Wf[l])
        shared[f'bv{l+1}'] = np.stack([bs[l], vs[l]])
    for i, nm in ((0, 'ld1'), (1, 'ld2'), (2, 'ld3')):
        g, bb = lds[i]
        shared[nm + 'gb'] = np.broadcast_to(g, (BC, g.size)).copy()
        shared[nm + 'bb'] = np.broadcast_to(bb, (BC, bb.size)).copy()
    selm = np.zeros((BC, NPACK * PKW), f32)
    for m in range(NPACK):
        for bl in range(2):
            selm[2 * m + bl, PKW * m + 50 * bl:PKW * m + 50 * bl + 50] = 1.0
    shared['sel'] = selm

    in_maps = []
    for c in range(NC_CORES):
        sl = slice(BC * c, BC * c + BC)
        d = dict(shared)
        d['bT'] = b_in[sl].transpose(2, 0, 1).reshape(M, P * BC).copy()
        d['y0T'] = y0[sl].transpose(2, 0, 1).reshape(N, P * BC).copy()
        d['y0n'] = y0[sl].reshape(NPACK * PKW, N).copy()
        d['U0n'] = U0[sl].reshape(NPACK * PKW, N).copy()
        d['d0n'] = d0[sl].reshape(NPACK * PKW, N).copy()
        GtT = np.zeros((NPACK, 102, PKW), f32)
        LtT = np.zeros((NPACK, PKW, PKW), f32)
        snbm = np.zeros((PKW, NPACK), f32)
        for m in range(NPACK):
            for bl in range(2):
                bg = BC * c + 2 * m + bl
                s, dd = edge[bg, 0], edge[bg, 1]
                cnt = np.zeros((P, P), np.int64)
                np.add.at(cnt, (dd, s), 1)
                deg = (cnt.sum(1) + 1).astype(f32)
                nb = cnt.sum(0).astype(f32)
                G = (cnt.astype(f32)
                     / np.sqrt(deg[:, None] * deg[None, :]).astype(f32))
                G[np.arange(P), np.arange(P)] += (f32(1.0) / deg)
                L = 2.0 * (np.diag(nb) - cnt.astype(f32))
                r0 = 50 * bl
                GtT[m, r0:r0 + 50, r0:r0 + 50] = G.T
                GtT[m, 101, r0:r0 + 50] = G.sum(1)
                LtT[m, r0:r0 + 50, r0:r0 + 50] = L.T.astype(f32)
                snbm[r0:r0 + 50, m] = nb
        GtT[:, 100, :] = 1.0
        d['GhatT'] = GtT[:, :PKW, :].reshape(NPACK * PKW, PKW).copy()
        d['GhatE'] = GtT[:, PKW:102, :].reshape(NPACK * 2, PKW).copy()
        d['LdT'] = LtT.reshape(NPACK * PKW, PKW).copy()
        d['snb'] = snbm
        in_maps.append(d)
    return in_maps, flags


def kernel(**inputs):
    in_maps, flags = _host_inputs(inputs)
    key = tuple(sorted(flags.items()))
    if key not in _NC_CACHE:
        _NC_CACHE[key] = build_nc(**flags)
    nc = _NC_CACHE[key]
    res = bass_utils.run_bass_kernel_spmd(nc, in_maps,
                                          core_ids=list(range(NC_CORES)))
    out = np.empty((K, B, P, N, 1), np.float32)
    for c in range(NC_CORES):
        ys = res.results[c]['Ys'].reshape(K, BC, P, N)
        out[:, BC * c:BC * c + BC] = ys[..., None]
    return out



# revision 3
# speedup vs baseline: 1.4540x; 1.4540x over previous
"""DLASSO-GNN-Hypernet ADMM forward on 8 Trainium2 NeuronCores (Bass/Tile).

Sharding: data-parallel over batch (8 samples/core). Graph message passing is
done as dense per-sample [50,50] operators built host-side from the integer
edge lists (count matrices + symmetric normalization). ADMM state math (AtAy,
grad, y/U/delta updates) stays fp32; the GCN encoder + dense hypernet run in
bf16 (weights, activations, and the streamed 26MB Wd1) — the hypernet only
produces the 4 per-sample step sizes, so bf16 error there is harmless.

Layouts per core (8 samples, 2-sample "packs", pack rows r = b_loc*50 + p):
  transposed activations: [feat, col] with col = b*50 + p   (feature on SBUF
  partitions; per-node column groups of 8 (stride 50) for per-node matmuls)
  natural ADMM states:    4 tiles [100, 200] (pack-stacked rows)
"""
import numpy as np

import concourse.bass as bass
import concourse.bacc as bacc
import concourse.tile as tile
import concourse.mybir as mybir
from concourse import bass_utils

F32 = mybir.dt.float32
BF16 = mybir.dt.bfloat16
F16 = mybir.dt.float16
AF = mybir.ActivationFunctionType
ALU = mybir.AluOpType

B, P, M, N, H, K = 64, 50, 100, 200, 128, 5
BC = 8                      # samples per core
NPACK, PKW = 4, 100         # packs of 2 samples -> 100 rows each
NC_CORES = 8
FOUT = [128, 256, 512, 512, 512]        # GCN layer output dims
FIN = [400, 128, 256, 512, 512]
LN_EPS = 1e-5
RING_BUFS = 5

KT400 = [(0, 128), (128, 72), (200, 128), (328, 72)]
KT200 = [(0, 128), (128, 72)]


def _kt(dim):
    if dim == 400:
        return KT400
    return [(s, min(128, dim - s)) for s in range(0, dim, 128)]


def _pk(ap_2d, pack):
    """[*, 400]-col AP (col = b*50+p) -> pack's contiguous 100 cols."""
    return ap_2d[:, PKW * pack:PKW * pack + PKW]


def _nd(ap_2d, p):
    """[*, 400]-col AP (col = b*50+p) -> node p's 8 sample-cols (stride 50)."""
    return ap_2d.rearrange("a (b p) -> a p b", p=P)[:, p, :]


def _newton_rsqrt(nc, pool, t_ap, shape, tag):
    """rstd = 1/sqrt(t) to ~1 ulp. ACT sqrt seed (loose LUT) + 2 Newton steps.
    r <- r*(1.5 - 0.5*t*r^2)."""
    s0 = pool.tile(shape, F32, tag=tag + "s0", name=tag + "s0")
    nc.scalar.activation(s0[:], t_ap, AF.Sqrt)
    r = pool.tile(shape, F32, tag=tag + "r", name=tag + "r")
    nc.vector.reciprocal(r[:], s0[:])
    tmp = pool.tile(shape, F32, tag=tag + "t", name=tag + "t")
    for _ in range(2):
        nc.vector.tensor_mul(tmp[:], r[:], r[:])
        nc.vector.tensor_mul(tmp[:], tmp[:], t_ap)
        nc.vector.tensor_scalar(tmp[:], tmp[:], -0.5, 1.5, ALU.mult, ALU.add)
        nc.vector.tensor_mul(r[:], r[:], tmp[:])
    return r


def build_nc(skip_bv=True, skip_ld=True, skip_bd=True):
    """skip_* : omit device work for bias/affine params that are all-zero /
    identity in the given inputs (decided host-side)."""
    nc = bacc.Bacc("TRN2", target_bir_lowering=False, debug=False,
                   enable_asserts=False, num_devices=NC_CORES)

    def din(name, shape, dt=F32):
        return nc.dram_tensor(name, list(shape), dt, kind="ExternalInput").ap()

    d_A0 = din("A0l", (M, P * N))
    d_bT = din("bT", (M, BC * P))
    d_y0T = din("y0T", (N, BC * P))
    d_y0n = din("y0n", (NPACK * PKW, N))
    d_U0n = din("U0n", (NPACK * PKW, N))
    d_d0n = din("d0n", (NPACK * PKW, N))
    d_Gt = din("GhatT", (NPACK * PKW, PKW), F16)
    d_Gte = din("GhatE", (NPACK * 2, PKW), F16)
    d_Lt = din("LdT", (NPACK * PKW, PKW))
    d_sel = din("sel", (BC, NPACK * PKW))
    d_snb = din("snb", (PKW, NPACK))
    d_id = din("ident", (128, 128))
    d_idb = din("identb", (128, 128), F16)
    d_W = [din(f"W{l+1}f", (FIN[l], FOUT[l]), F16) for l in range(5)]
    d_bv = [din(f"bv{l+1}", (2, FOUT[l]), F16) for l in range(5)]
    d_Wd1 = din("Wd1", (P * 512, 512), F16)
    d_Wd2 = din("Wd2", (512, 256), F16)
    d_Wd3 = din("Wd3", (256, 128), F16)
    d_Wfc = din("Wfc", (128, 4), F16)
    d_bdr = [din(n, (1, f)) for n, f in
             (("bd1r", 512), ("bd2r", 256), ("bd3r", 128), ("bfcr", 4))]
    d_ldg = [din(n, (BC, f)) for n, f in
             (("ld1gb", 512), ("ld2gb", 256), ("ld3gb", 128))]
    d_ldb = [din(n, (BC, f)) for n, f in
             (("ld1bb", 512), ("ld2bb", 256), ("ld3bb", 128))]
    d_g5 = din("gs5c", (128, 4))
    d_b5 = din("be5c", (128, 4))
    d_lng = din("lngc", (128, 4))
    d_lnb = din("lnbc", (128, 4))
    d_out = nc.dram_tensor("Ys", [K, NPACK * PKW, N], F32,
                           kind="ExternalOutput").ap()

    with tile.TileContext(nc) as tc:
        from contextlib import ExitStack
        es = ExitStack()
        with es:
            cp = es.enter_context(tc.tile_pool(name="consts", bufs=1))

            def load(dram_ap, shape, tag, dt=F32):
                t = cp.tile(list(shape), dt, tag=tag, name=tag)
                nc.sync.dma_start(out=t[:], in_=dram_ap)
                return t

            ident = load(d_id, (128, 128), "ident")
            identb = load(d_idb, (128, 128), "identb", F16)
            Gt = [load(d_Gt[PKW * m:PKW * m + PKW, :], (PKW, PKW), f"Gt{m}",
                       F16)
                  for m in range(NPACK)]
            Gte = [load(d_Gte[2 * m:2 * m + 2, :], (2, PKW), f"Gte{m}", F16)
                   for m in range(NPACK)]
            Lt = [load(d_Lt[PKW * m:PKW * m + PKW, :], (PKW, PKW), f"Lt{m}")
                  for m in range(NPACK)]
            sel = load(d_sel, (BC, NPACK * PKW), "sel")
            snb = load(d_snb, (PKW, NPACK), "snb")
            Wt = {}
            for l in range(5):
                for ki, (ks, kz) in enumerate(_kt(FIN[l])):
                    Wt[(l, ki)] = load(d_W[l][ks:ks + kz, :], (kz, FOUT[l]),
                                       f"W{l}_{ki}", F16)
            bv = (None if skip_bv else
                  [load(d_bv[l], (2, FOUT[l]), f"bv{l}", F16)
                   for l in range(5)])
            Wd2t = [load(d_Wd2[s:s + z, :], (z, 256), f"wd2_{s}", F16)
                    for s, z in _kt(512)]
            Wd3t = [load(d_Wd3[s:s + z, :], (z, 128), f"wd3_{s}", F16)
                    for s, z in _kt(256)]
            Wfct = load(d_Wfc, (128, 4), "wfc", F16)
            if skip_bd:
                bdr = [None, None, None, load(d_bdr[3], (1, 4), "bdr3")]
            else:
                bdr = [load(d_bdr[i], d_bdr[i].shape, f"bdr{i}")
                       for i in range(4)]
            ldg = ldb = None
            if not skip_ld:
                ldg = [load(d_ldg[i], d_ldg[i].shape, f"ldg{i}")
                       for i in range(3)]
                ldb = [load(d_ldb[i], d_ldb[i].shape, f"ldb{i}")
                       for i in range(3)]
            g5c = load(d_g5, (128, 4), "g5c")
            b5c = load(d_b5, (128, 4), "b5c")
            lngc = load(d_lng, (128, 4), "lngc")
            lnbc = load(d_lnb, (128, 4), "lnbc")

            ones_col = cp.tile([128, 1], F16, tag="ones_col")
            nc.vector.memset(ones_col[:], 1.0)
            ones_row = cp.tile([1, 128], F32, tag="ones_row")
            nc.vector.memset(ones_row[:], 1.0)
            ones_cb = cp.tile([128, 1], BF16, tag="ones_cb")
            nc.vector.memset(ones_cb[:], 1.0)

            yT = [cp.tile([128, BC * P], F32, tag="yT0", name="yT0"),
                  cp.tile([72, BC * P], F32, tag="yT1", name="yT1")]
            nc.sync.dma_start(out=yT[0][:], in_=d_y0T[0:128, :])
            nc.sync.dma_start(out=yT[1][:], in_=d_y0T[128:200, :])
            y_nat, U_nat, dl_nat, Atb_nat = [
                [cp.tile([PKW, N], F32, tag=f"{nm}{m}", name=f"{nm}{m}")
                 for m in range(NPACK)]
                for nm in ("y", "U", "dl", "Atb")]
            for m in range(NPACK):
                for tl, src in ((y_nat, d_y0n), (U_nat, d_U0n), (dl_nat, d_d0n)):
                    nc.sync.dma_start(out=tl[m][:],
                                      in_=src[PKW * m:PKW * m + PKW, :])
            xnt = [cp.tile([z, BC * P], F32, tag=f"xnt{i}", name=f"xnt{i}")
                   for i, (s, z) in enumerate(KT400)]
            xnb = [cp.tile([z, BC * P], F16, tag=f"xnb{i}", name=f"xnb{i}")
                   for i, (s, z) in enumerate(KT400)]
            hT = [cp.tile([128, BC * P], F16, tag=f"h{i}", name=f"h{i}")
                  for i in range(4)]          # also holds enc^T after LN
            AtA = [cp.tile([128, P * N], F32, tag="ata0", name="ata0"),
                   cp.tile([72, P * N], F32, tag="ata1", name="ata1")]

            # ---------------- setup: AtA = A0^T A0, Atb = A0^T b ----------
            with tc.tile_pool(name="setup", bufs=1) as sp, \
                 tc.tile_pool(name="ps_setup", bufs=2, space="PSUM") as pps:
                bT = sp.tile([M, BC * P], F32, tag="bTs")
                nc.sync.dma_start(out=bT[:], in_=d_bT)
                HP = P * N // 2
                for half in range(2):
                    A0 = sp.tile([M, HP], F32, tag="A0", name="A0")
                    nc.sync.dma_start(out=A0[:],
                                      in_=d_A0[:, half * HP:(half + 1) * HP])
                    for p in range(half * (P // 2), (half + 1) * (P // 2)):
                        a0p = A0[:, N * p - half * HP:N * p - half * HP + N]
                        for i, (s, z) in enumerate(KT200):
                            ps = pps.tile([z, N], F32, tag="sat", name="sat")
                            nc.tensor.matmul(ps[:], lhsT=a0p[:, s:s + z],
                                             rhs=a0p, start=True, stop=True)
                            nc.vector.tensor_copy(AtA[i][:, N * p:N * p + N],
                                                  ps[:])
                            pb = pps.tile([z, BC], F32, tag="sab", name="sab")
                            nc.tensor.matmul(pb[:], lhsT=a0p[:, s:s + z],
                                             rhs=_nd(bT[:], p),
                                             start=True, stop=True)
                            nc.vector.tensor_copy(
                                _nd(xnt[2 + i][:], p), pb[:])
                for m in range(NPACK):
                    for i, (s, z) in enumerate(KT200):
                        pt = pps.tile([PKW, 128], F32, tag="satr", name="satr")
                        nc.tensor.transpose(pt[:, :z], _pk(xnt[2 + i][:], m),
                                            ident[:z, :z])
                        nc.vector.tensor_copy(Atb_nat[m][:, s:s + z],
                                              pt[:, :z])
                for i in (2, 3):
                    nc.scalar.copy(xnb[i][:], xnt[i][:])

            ring = es.enter_context(tc.tile_pool(name="ring", bufs=RING_BUFS))

            # ---------------- K ADMM steps --------------------------------
            for k in range(K):
                sw = tc.alloc_tile_pool(name=f"sw{k}", bufs=1)
                # -- AtAy^T into xnt[0], xnt[1] --
                with tc.tile_pool(name=f"ps_aty{k}", bufs=2,
                                  space="PSUM") as pp:
                    for p in range(P):
                        for i, (s, z) in enumerate(KT200):
                            ps = pp.tile([z, BC], F32, tag=f"aty{i}",
                                         name="aty")
                            for j, (js, jz) in enumerate(KT200):
                                nc.tensor.matmul(
                                    ps[:],
                                    lhsT=AtA[j][:, N * p + s:N * p + s + z],
                                    rhs=_nd(yT[j][:], p),
                                    start=(j == 0), stop=(j == 1))
                            nc.vector.tensor_copy(
                                _nd(xnt[i][:], p), ps[:])
                for i in (0, 1):
                    nc.scalar.copy(xnb[i][:], xnt[i][:])
                # -- g = AtAy - Atb (natural, via PE transpose) --
                g_t = [sw.tile([PKW, N], F32, tag="g", name=f"g{m}", bufs=4)
                       for m in range(NPACK)]
                with tc.tile_pool(name=f"ps_atr{k}", bufs=2,
                                  space="PSUM") as pp:
                    for m in range(NPACK):
                        for i, (s, z) in enumerate(KT200):
                            pt = pp.tile([PKW, 128], F32, tag="atr",
                                         name="atr")
                            nc.tensor.transpose(pt[:, :z], _pk(xnt[i][:], m),
                                                ident[:z, :z])
                            nc.vector.tensor_sub(g_t[m][:, s:s + z],
                                                 pt[:, :z],
                                                 Atb_nat[m][:, s:s + z])

                # -- GCN: 5 layers, transposed chain (bf16) --
                cur = xnb
                with tc.tile_pool(name=f"ps_gcn{k}", bufs=2,
                                  space="PSUM") as pp, \
                     tc.tile_pool(name=f"gcnw{k}", bufs=2) as gw:
                    for l in range(5):
                        fo = FOUT[l]
                        nxt = (hT if l == 4 else
                               [gw.tile([128, BC * P], F16, tag=f"xt{i}",
                                        name=f"xt{i}")
                                for i in range((fo + 127) // 128)])
                        for m in range(NPACK):
                            psF = pp.tile([PKW, 512], F32, tag="F", name="F")
                            kts = _kt(FIN[l])
                            for ki, (ks, kz) in enumerate(kts):
                                nc.tensor.matmul(
                                    psF[:, :fo], lhsT=_pk(cur[ki][:], m),
                                    rhs=Wt[(l, ki)][:],
                                    start=(ki == 0), stop=(ki == len(kts) - 1))
                            Fsb = gw.tile([PKW, 512], F16, tag="Fsb",
                                          name="Fsb")
                            nc.scalar.copy(Fsb[:, :fo], psF[:, :fo])
                            for mi, (ms, mz) in enumerate(_kt(fo)):
                                psZ = pp.tile([128, PKW], F32, tag="Z",
                                              name="Z")
                                nc.tensor.matmul(
                                    psZ[:mz, :], lhsT=Fsb[:, ms:ms + mz],
                                    rhs=Gt[m][:], start=True, stop=skip_bv)
                                if not skip_bv:
                                    nc.tensor.matmul(
                                        psZ[:mz, :],
                                        lhsT=bv[l][:, ms:ms + mz],
                                        rhs=Gte[m][:], start=False, stop=True)
                                nc.scalar.activation(
                                    _pk(nxt[mi][:mz, :], m), psZ[:mz, :],
                                    AF.Lrelu, alpha=0.01)
                        cur = nxt

                # -- bn5 + LayerNorm over features (transposed) --
                with tc.tile_pool(name=f"ps_ln{k}", bufs=2, space="PSUM") as pp, \
                     tc.tile_pool(name=f"lnw{k}", bufs=1) as lw:
                    for i in range(4):
                        nc.vector.tensor_scalar(
                            hT[i][:], hT[i][:], g5c[:, i:i + 1],
                            b5c[:, i:i + 1], ALU.mult, ALU.add)
                    psmu = pp.tile([1, BC * P], F32, tag="mu", name="psmu")
                    for i in range(4):
                        nc.tensor.matmul(psmu[:], lhsT=ones_col[:],
                                         rhs=hT[i][:], start=(i == 0),
                                         stop=(i == 3))
                    mu_r = lw.tile([1, BC * P], F32, tag="mu_r")
                    nc.scalar.mul(mu_r[:], psmu[:], 1.0 / 512.0)
                    pssq = pp.tile([1, BC * P], F32, tag="sq", name="pssq")
                    for i in range(4):
                        hsq = lw.tile([128, BC * P], BF16, tag="hsq",
                                      name="hsq", bufs=2)
                        nc.vector.tensor_mul(hsq[:], hT[i][:], hT[i][:])
                        nc.tensor.matmul(pssq[:], lhsT=ones_cb[:], rhs=hsq[:],
                                         start=(i == 0), stop=(i == 3))
                    var_r = lw.tile([1, BC * P], F32, tag="var_r")
                    nc.scalar.mul(var_r[:], pssq[:], 1.0 / 512.0)
                    t_r = lw.tile([1, BC * P], F32, tag="t_r")
                    nc.vector.tensor_mul(t_r[:], mu_r[:], mu_r[:])
                    nc.vector.tensor_sub(var_r[:], var_r[:], t_r[:])
                    nc.vector.tensor_scalar_add(var_r[:], var_r[:], LN_EPS)
                    rs_r = _newton_rsqrt(nc, lw, var_r[:], [1, BC * P], "lnr")
                    bco = {}
                    for nm, row in (("mu", mu_r), ("rs", rs_r)):
                        psb = pp.tile([128, BC * P], F32, tag="bc" + nm,
                                      name="psb")
                        nc.tensor.matmul(psb[:], lhsT=ones_row[:], rhs=row[:],
                                         start=True, stop=True)
                        sb = lw.tile([128, BC * P], F16, tag="bcs" + nm,
                                     name="bcs")
                        nc.vector.tensor_copy(sb[:], psb[:])
                        bco[nm] = sb
                    for i in range(4):
                        nc.vector.tensor_sub(hT[i][:], hT[i][:], bco["mu"][:])
                        nc.vector.tensor_mul(hT[i][:], hT[i][:], bco["rs"][:])
                        nc.vector.tensor_scalar(
                            hT[i][:], hT[i][:], lngc[:, i:i + 1],
                            lnbc[:, i:i + 1], ALU.mult, ALU.add)

                # -- dense hyp-net (Wd1 streamed from HBM, bf16) --
                scal, nal = [], []
                with tc.tile_pool(name=f"ps_d{k}", bufs=1, space="PSUM") as pz, \
                     tc.tile_pool(name=f"ps_dt{k}", bufs=2, space="PSUM") as pt, \
                     tc.tile_pool(name=f"dw{k}", bufs=1) as dw:
                    psz1 = pz.tile([BC, 512], F32, tag="z1", name="psz1")
                    for t in range(P):
                        rg = ring.tile([128, 4 * 512], F16, tag="wd1",
                                       name="wd1")
                        src = d_Wd1[512 * t:512 * t + 512, :].rearrange(
                            "(c r) n -> r c n", r=128)
                        nc.sync.dma_start(
                            out=rg[:].rearrange("r (c n) -> r c n", c=4),
                            in_=src)
                        for c in range(4):
                            nc.tensor.matmul(
                                psz1[:], lhsT=_nd(hT[c][:], t),
                                rhs=rg[:, 512 * c:512 * c + 512],
                                start=(t == 0 and c == 0),
                                stop=(t == P - 1 and c == 3 and skip_bd))
                    if not skip_bd:
                        nc.tensor.matmul(psz1[:], lhsT=ones_row[:, :BC],
                                         rhs=bdr[0][:], start=False, stop=True)

                    def dense_ln_lrelu(ps_ap, fdim, li):
                        zr = dw.tile([BC, 512], F32, tag="d_zr", name="zr")
                        nc.vector.tensor_copy(zr[:, :fdim], ps_ap)
                        mu = dw.tile([BC, 1], F32, tag="d_mu", name="mu")
                        nc.vector.tensor_reduce(mu[:], zr[:, :fdim],
                                                mybir.AxisListType.X, ALU.add)
                        nc.vector.tensor_scalar_mul(mu[:], mu[:], 1.0 / fdim)
                        sq = dw.tile([BC, 512], F32, tag="d_sq", name="sq")
                        nc.vector.tensor_mul(sq[:, :fdim], zr[:, :fdim],
                                             zr[:, :fdim])
                        vr = dw.tile([BC, 1], F32, tag="d_vr", name="vr")
                        nc.vector.tensor_reduce(vr[:], sq[:, :fdim],
                                                mybir.AxisListType.X, ALU.add)
                        nc.vector.tensor_scalar_mul(vr[:], vr[:], 1.0 / fdim)
                        mm = dw.tile([BC, 1], F32, tag="d_mm", name="mm")
                        nc.vector.tensor_mul(mm[:], mu[:], mu[:])
                        nc.vector.tensor_sub(vr[:], vr[:], mm[:])
                        nc.vector.tensor_scalar_add(vr[:], vr[:], LN_EPS)
                        rs = _newton_rsqrt(nc, dw, vr[:], [BC, 1], "dn")
                        z = dw.tile([BC, 512], F16, tag="d_z", name="z")
                        nc.vector.tensor_scalar(z[:, :fdim], zr[:, :fdim],
                                                mu[:], rs[:],
                                                ALU.subtract, ALU.mult)
                        if not skip_ld:
                            nc.vector.tensor_mul(z[:, :fdim], z[:, :fdim],
                                                 ldg[li][:, :fdim])
                            nc.vector.tensor_add(z[:, :fdim], z[:, :fdim],
                                                 ldb[li][:, :fdim])
                        nc.vector.scalar_tensor_tensor(
                            z[:, :fdim], z[:, :fdim], 0.01, z[:, :fdim],
                            ALU.mult, ALU.max)
                        return z

                    def transpose_cols(z_sb, fdim):
                        outs = []
                        for ci, (cs, cz) in enumerate(_kt(fdim)):
                            pty = pt.tile([128, BC], F16, tag="zt",
                                          name="pty")
                            nc.tensor.transpose(pty[:cz, :],
                                                z_sb[:, cs:cs + cz],
                                                identb[:BC, :BC])
                            zz = dw.tile([128, BC], F16, tag="d_zt", bufs=4,
                                         name=f"zz{ci}")
                            nc.vector.tensor_copy(zz[:cz, :], pty[:cz, :])
                            outs.append((zz, cz))
                        return outs

                    z1 = dense_ln_lrelu(psz1[:], 512, 0)
                    z1t = transpose_cols(z1, 512)
                    psz2 = pz.tile([BC, 256], F32, tag="z2", name="psz2")
                    for ci, (zz, cz) in enumerate(z1t):
                        nc.tensor.matmul(psz2[:], lhsT=zz[:cz, :],
                                         rhs=Wd2t[ci][:], start=(ci == 0),
                                         stop=(ci == 3 and skip_bd))
                    if not skip_bd:
                        nc.tensor.matmul(psz2[:], lhsT=ones_row[:, :BC],
                                         rhs=bdr[1][:], start=False, stop=True)
                    z2 = dense_ln_lrelu(psz2[:], 256, 1)
                    z2t = transpose_cols(z2, 256)
                    psz3 = pz.tile([BC, 128], F32, tag="z3", name="psz3")
                    for ci, (zz, cz) in enumerate(z2t):
                        nc.tensor.matmul(psz3[:], lhsT=zz[:cz, :],
                                         rhs=Wd3t[ci][:], start=(ci == 0),
                                         stop=(ci == 1 and skip_bd))
                    if not skip_bd:
                        nc.tensor.matmul(psz3[:], lhsT=ones_row[:, :BC],
                                         rhs=bdr[2][:], start=False, stop=True)
                    z3 = dense_ln_lrelu(psz3[:], 128, 2)
                    z3t = transpose_cols(z3, 128)
                    psfc = pz.tile([BC, 4], F32, tag="fc", name="psfc")
                    nc.tensor.matmul(psfc[:], lhsT=z3t[0][0][:128, :],
                                     rhs=Wfct[:], start=True, stop=False)
                    nc.tensor.matmul(psfc[:], lhsT=ones_row[:, :BC],
                                     rhs=bdr[3][:], start=False, stop=True)
                    hyp = dw.tile([BC, 4], F32, tag="hyp", name="hyp")
                    nc.scalar.activation(hyp[:], psfc[:], AF.Sigmoid)
                    nc.vector.tensor_scalar(hyp[:], hyp[:], 0.9999, 1e-4,
                                            ALU.min, ALU.max)
                    for m in range(NPACK):
                        pss = pt.tile([PKW, 4], F32, tag="scal", name="pss")
                        nc.tensor.matmul(pss[:],
                                         lhsT=sel[:, PKW * m:PKW * m + PKW],
                                         rhs=hyp[:], start=True, stop=True)
                        sc = sw.tile([PKW, 4], F32, tag="sc", bufs=4,
                                     name=f"sc{m}")
                        nc.vector.tensor_copy(sc[:], pss[:])
                        na = sw.tile([PKW, 1], F32, tag="na", bufs=4,
                                     name=f"na{m}")
                        nc.vector.tensor_scalar_mul(na[:], sc[:, 0:1], -1.0)
                        scal.append(sc)
                        nal.append(na)

                # -- ADMM update (natural layout) --
                with tc.tile_pool(name=f"ps_ad{k}", bufs=2, space="PSUM") as pp, \
                     tc.tile_pool(name=f"adw{k}", bufs=2) as aw:
                    for m in range(NPACK):
                        g = g_t[m]
                        sg = aw.tile([PKW, N], F32, tag="sg", name="sg")
                        nc.scalar.activation(sg[:], y_nat[m][:], AF.Sign)
                        nc.vector.scalar_tensor_tensor(
                            g[:], sg[:], scal[m][:, 1:2], g[:],
                            ALU.mult, ALU.add)
                        nc.vector.scalar_tensor_tensor(
                            g[:], U_nat[m][:], snb[:, m:m + 1], g[:],
                            ALU.mult, ALU.add)
                        nc.vector.scalar_tensor_tensor(
                            g[:], dl_nat[m][:], scal[m][:, 2:3], g[:],
                            ALU.mult, ALU.add)
                        nc.vector.tensor_scalar(g[:], g[:], 10.0, -10.0,
                                                ALU.min, ALU.max)
                        nc.vector.scalar_tensor_tensor(
                            y_nat[m][:], g[:], nal[m][:], y_nat[m][:],
                            ALU.mult, ALU.add)
                        nc.vector.tensor_scalar(y_nat[m][:], y_nat[m][:],
                                                100.0, -100.0,
                                                ALU.min, ALU.max)
                        nc.sync.dma_start(
                            out=d_out[k, PKW * m:PKW * m + PKW, :],
                            in_=y_nat[m][:])
                        psd = pp.tile([PKW, N], F32, tag="dl", name="psd")
                        nc.tensor.matmul(psd[:], lhsT=Lt[m][:],
                                         rhs=y_nat[m][:], start=True,
                                         stop=True)
                        nc.vector.tensor_scalar(dl_nat[m][:], psd[:],
                                                20.0, -20.0, ALU.min, ALU.max)
                        nc.vector.scalar_tensor_tensor(
                            U_nat[m][:], dl_nat[m][:], scal[m][:, 3:4],
                            U_nat[m][:], ALU.mult, ALU.add)
                        nc.vector.tensor_scalar(U_nat[m][:], U_nat[m][:],
                                                100.0, -100.0,
                                                ALU.min, ALU.max)
                        if k < K - 1:
                            for j, (js, jz) in enumerate(KT200):
                                pty = pp.tile([128, PKW], F32, tag="ytr",
                                              name="pty2")
                                nc.tensor.transpose(
                                    pty[:jz, :], y_nat[m][:, js:js + jz],
                                    ident[:PKW, :PKW])
                                nc.vector.tensor_copy(
                                    _pk(yT[j][:], m), pty[:jz, :])
                sw.release()
    nc.compile()
    return nc


_NC_CACHE = {}


def _host_inputs(inputs):
    """Build all per-core DRAM arrays (numpy). Returns (in_maps, flags)."""
    f32 = np.float32
    BN_SCALE = f32(1.0) / np.sqrt(f32(1.0) + f32(1e-5))
    b_in = np.ascontiguousarray(np.asarray(inputs['b'], f32)[..., 0])
    A0 = np.ascontiguousarray(np.asarray(inputs['A'], f32)[0])
    edge = np.asarray(inputs['edge_index'])
    y0 = np.ascontiguousarray(np.asarray(inputs['y0'], f32)[..., 0])
    U0 = np.ascontiguousarray(np.asarray(inputs['U0'], f32)[..., 0])
    d0 = np.ascontiguousarray(np.asarray(inputs['delta0'], f32)[..., 0])

    Ws = [np.asarray(inputs['W%d' % i], f32) for i in range(1, 6)]
    bs = [np.asarray(inputs['b%d' % i], f32) for i in range(1, 6)]
    gs = [np.asarray(inputs['g%d' % i], f32) * BN_SCALE for i in range(1, 6)]
    bes = [np.asarray(inputs['be%d' % i], f32) for i in range(1, 6)]
    Wf = [Ws[0]] + [gs[l - 1][:, None] * Ws[l] for l in range(1, 5)]
    vs = [np.zeros(FOUT[0], f32)] + [(bes[l - 1] @ Ws[l]).astype(f32)
                                     for l in range(1, 5)]
    lds = [(np.asarray(inputs['ld%dg' % i], f32),
            np.asarray(inputs['ld%db' % i], f32)) for i in (1, 2, 3)]
    bds = [np.asarray(inputs['bd%d' % i], f32) for i in (1, 2, 3)]

    flags = dict(
        skip_bv=bool(all(np.all(bs[l] == 0) and np.all(vs[l] == 0)
                         for l in range(5))),
        skip_ld=bool(all(np.all(g == 1) and np.all(bb == 0)
                         for g, bb in lds)),
        skip_bd=bool(all(np.all(bd == 0) for bd in bds)),
    )

    shared = {'A0l': A0.transpose(1, 0, 2).reshape(M, P * N).copy(),
              'ident': np.eye(128, dtype=f32),
              'identb': np.eye(128, dtype=np.float16),
              'Wd1': np.asarray(inputs['Wd1'], f32).astype(np.float16),
              'Wd2': np.asarray(inputs['Wd2'], f32).astype(np.float16),
              'Wd3': np.asarray(inputs['Wd3'], f32).astype(np.float16),
              'Wfc': np.asarray(inputs['Wfc'], f32).astype(np.float16),
              'bd1r': bds[0][None, :], 'bd2r': bds[1][None, :],
              'bd3r': bds[2][None, :],
              'bfcr': np.asarray(inputs['bfc'], f32)[None, :],
              'gs5c': np.ascontiguousarray(gs[4].reshape(4, 128).T),
              'be5c': np.ascontiguousarray(bes[4].reshape(4, 128).T),
              'lngc': np.ascontiguousarray(
                  np.asarray(inputs['ln_g'], f32).reshape(4, 128).T),
              'lnbc': np.ascontiguousarray(
                  np.asarray(inputs['ln_b'], f32).reshape(4, 128).T)}
    for l in range(5):
        shared[f'W{l+1}f'] = np.ascontiguousarray(Wf[l]).astype(np.float16)
        shared[f'bv{l+1}'] = np.stack([bs[l], vs[l]]).astype(np.float16)
    for i, nm in ((0, 'ld1'), (1, 'ld2'), (2, 'ld3')):
        g, bb = lds[i]
        shared[nm + 'gb'] = np.broadcast_to(g, (BC, g.size)).copy()
        shared[nm + 'bb'] = np.broadcast_to(bb, (BC, bb.size)).copy()
    selm = np.zeros((BC, NPACK * PKW), f32)
    for m in range(NPACK):
        for bl in range(2):
            selm[2 * m + bl, PKW * m + 50 * bl:PKW * m + 50 * bl + 50] = 1.0
    shared['sel'] = selm

    in_maps = []
    for c in range(NC_CORES):
        sl = slice(BC * c, BC * c + BC)
        d = dict(shared)
        d['bT'] = b_in[sl].transpose(2, 0, 1).reshape(M, P * BC).copy()
        d['y0T'] = y0[sl].transpose(2, 0, 1).reshape(N, P * BC).copy()
        d['y0n'] = y0[sl].reshape(NPACK * PKW, N).copy()
        d['U0n'] = U0[sl].reshape(NPACK * PKW, N).copy()
        d['d0n'] = d0[sl].reshape(NPACK * PKW, N).copy()
        GtT = np.zeros((NPACK, 102, PKW), f32)
        LtT = np.zeros((NPACK, PKW, PKW), f32)
        snbm = np.zeros((PKW, NPACK), f32)
        for m in range(NPACK):
            for bl in range(2):
                bg = BC * c + 2 * m + bl
                s, dd = edge[bg, 0], edge[bg, 1]
                cnt = np.zeros((P, P), np.int64)
                np.add.at(cnt, (dd, s), 1)
                deg = (cnt.sum(1) + 1).astype(f32)
                nb = cnt.sum(0).astype(f32)
                G = (cnt.astype(f32)
                     / np.sqrt(deg[:, None] * deg[None, :]).astype(f32))
                G[np.arange(P), np.arange(P)] += (f32(1.0) / deg)
                L = 2.0 * (np.diag(nb) - cnt.astype(f32))
                r0 = 50 * bl
                GtT[m, r0:r0 + 50, r0:r0 + 50] = G.T
                GtT[m, 101, r0:r0 + 50] = G.sum(1)
                LtT[m, r0:r0 + 50, r0:r0 + 50] = L.T.astype(f32)
                snbm[r0:r0 + 50, m] = nb
        GtT[:, 100, :] = 1.0
        d['GhatT'] = GtT[:, :PKW, :].reshape(NPACK * PKW, PKW).astype(np.float16)
        d['GhatE'] = GtT[:, PKW:102, :].reshape(NPACK * 2, PKW).astype(np.float16)
        d['LdT'] = LtT.reshape(NPACK * PKW, PKW).copy()
        d['snb'] = snbm
        in_maps.append(d)
    return in_maps, flags


def kernel(**inputs):
    in_maps, flags = _host_inputs(inputs)
    key = tuple(sorted(flags.items()))
    if key not in _NC_CACHE:
        _NC_CACHE[key] = build_nc(**flags)
    nc = _NC_CACHE[key]
    res = bass_utils.run_bass_kernel_spmd(nc, in_maps,
                                          core_ids=list(range(NC_CORES)))
    out = np.empty((K, B, P, N, 1), np.float32)
    for c in range(NC_CORES):
        ys = res.results[c]['Ys'].reshape(K, BC, P, N)
        out[:, BC * c:BC * c + BC] = ys[..., None]
    return out


# revision 18
# speedup vs baseline: 2.0250x; 1.3927x over previous
"""DLASSO-GNN-Hypernet ADMM forward on 8 Trainium2 NeuronCores (Bass/Tile).

Sharding: data-parallel over batch (8 samples/core) for the GCN + ADMM state;
the 26MB fp16 Wd1 hypernet matrix is contraction-sharded across the 8 cores
(1/8 resident in each core's SBUF) — per step the encodings are exchanged
with an AllToAll, each core computes a 1/8-contraction partial of z1 for all
64 samples, and a ReduceScatter returns the per-sample sums.

Precision: ADMM state math (grad assembly, y/U/delta updates) is fp32;
all matmuls (GCN, hypernet, AtAy, L) run in fp16 on the PE (1 cycle/row vs
fp32's 4) with fp32 PSUM accumulation. Graph message passing uses dense
per-sample [50,50] operators built host-side from the integer edge lists.

Layouts per core (8 samples; natural row index r = p*8 + b):
  natural ADMM state: 4 node-group tiles [RG, 200], groups of 13/13/12/12
  nodes (rows 104/104/96/96) so AtAy matmuls write [8,200] per-node slices.
  transposed activations: [feat, col] with col = r; GCN aggregation is done
  transposed (H^T = sum_g' Fsb_g'^T @ Ghat_blk) so each layer alternates
  natural F / transposed H without explicit transposes.
"""
import numpy as np

import concourse.bass as bass
import concourse.bacc as bacc
import concourse.tile as tile
import concourse.mybir as mybir
from concourse import bass_utils
from concourse.tile_rust import add_dep_helper

F32 = mybir.dt.float32
BF16 = mybir.dt.bfloat16
F16 = mybir.dt.float16
AF = mybir.ActivationFunctionType
ALU = mybir.AluOpType

B, P, M, N, H, K = 64, 50, 100, 200, 128, 5
BC = 8                      # samples per core
NC_CORES = 8
FOUT = [128, 256, 512, 512, 512]        # GCN layer output dims
FIN = [400, 128, 256, 512, 512]
LN_EPS = 1e-5

NG = [16, 16, 16, 2]                    # nodes per group (quad-aligned)
PG0 = [0, 16, 32, 48]                   # node offset per group
RG = [128, 128, 128, 16]                # rows per group (= 8 * NG)
RG0 = [0, 128, 256, 384]                # row offset per group
NSLOT = 7                               # Wd1 shard t-slots (50 = 8*6 + 2)

KT400 = [(0, 128), (128, 72), (200, 128), (328, 72)]
KT200 = [(0, 128), (128, 72)]

STREAM_WD1 = False                      # True: HBM-stream Wd1 (no collectives)
RING_BUFS = 5


def _kt(dim):
    if dim == 400:
        return KT400
    return [(s, min(128, dim - s)) for s in range(0, dim, 128)]


def _grp(p):
    for g in range(4):
        if p < PG0[g] + NG[g]:
            return g
    raise AssertionError


def _newton_rsqrt(nc, pool, t_ap, shape, tag):
    """rstd = 1/sqrt(t): ACT sqrt + DVE reciprocal. LN rstd errors are pure
    per-sample scale errors and wash out through the downstream LN chain."""
    s0 = pool.tile(shape, F32, tag=tag + "s0", name=tag + "s0")
    nc.scalar.activation(s0[:], t_ap, AF.Sqrt)
    r = pool.tile(shape, F32, tag=tag + "r", name=tag + "r")
    nc.vector.reciprocal(r[:], s0[:])
    return r


def build_nc(skip_bv=True, skip_ld=True, skip_bd=True):
    nc = bacc.Bacc("TRN2", target_bir_lowering=False, debug=False,
                   enable_asserts=False, num_devices=NC_CORES)

    def din(name, shape, dt=F32):
        return nc.dram_tensor(name, list(shape), dt, kind="ExternalInput").ap()

    d_A0 = din("A0l", (M, P * N))                 # fp32, for Atb
    d_A0h = din("A0h", (M, P * N), F16)           # fp16, for AtA
    d_bT = din("bT", (M, BC * P))
    d_y0T = din("y0T", (N, BC * P), F16)
    d_y0n = din("y0n", (P * BC, N))
    d_U0n = din("U0n", (P * BC, N))
    d_d0n = din("d0n", (P * BC, N))
    d_GhB = din("GhB", (P * BC, P * BC), F16)     # transposed agg operator
    d_GteN = din("GteN", (2, P * BC), F16)
    d_Lh = din("Lh", (P * BC, P * BC), F16)       # transposed Laplacian op
    d_sel = din("sel", (BC, P * BC))
    d_snb = din("snb", (128, 4))
    d_id = din("ident", (128, 128))
    d_idh = din("identh", (128, 128), F16)
    d_W = [din(f"W{l+1}f", (FIN[l], FOUT[l]), F16) for l in range(5)]
    d_bv = [din(f"bv{l+1}", (2, FOUT[l]), F16) for l in range(5)]
    if STREAM_WD1:
        d_Wd1 = din("Wd1", (P * 512, 512), F16)
    else:
        d_Wd1s = din("Wd1s", (128, NSLOT * 4 * 512), F16)
    d_Wd2 = din("Wd2", (512, 256), F16)
    d_Wd3 = din("Wd3", (256, 128), F16)
    d_Wfc = din("Wfc", (128, 4), F16)
    d_bdr = [din(n, (1, f)) for n, f in
             (("bd1r", 512), ("bd2r", 256), ("bd3r", 128), ("bfcr", 4))]
    d_ldg = [din(n, (BC, f)) for n, f in
             (("ld1gb", 512), ("ld2gb", 256), ("ld3gb", 128))]
    d_ldb = [din(n, (BC, f)) for n, f in
             (("ld1bb", 512), ("ld2bb", 256), ("ld3bb", 128))]
    d_g5 = din("gs5c", (128, 4))
    d_b5 = din("be5c", (128, 4))
    d_lng = din("lngc", (128, 4))
    d_lnb = din("lnbc", (128, 4))
    d_out = nc.dram_tensor("Ys", [K, P * BC, N], F32,
                           kind="ExternalOutput").ap()

    with tile.TileContext(nc) as tc:
        from contextlib import ExitStack
        es = ExitStack()
        with es:
            cp = es.enter_context(tc.tile_pool(name="consts", bufs=1))

            def load(dram_ap, shape, tag, dt=F32):
                t = cp.tile(list(shape), dt, tag=tag, name=tag)
                nc.sync.dma_start(out=t[:], in_=dram_ap)
                return t

            ident = load(d_id, (128, 128), "ident")
            identh = load(d_idh, (128, 128), "identh", F16)
            GhB = [load(d_GhB[RG0[g]:RG0[g] + RG[g], :], (RG[g], P * BC),
                        f"GhB{g}", F16) for g in range(4)]
            GteN = (None if skip_bv else
                    load(d_GteN, (2, P * BC), "GteN", F16))
            Lh = [load(d_Lh[RG0[g]:RG0[g] + RG[g], :], (RG[g], P * BC),
                       f"Lh{g}", F16) for g in range(4)]
            sel = load(d_sel, (BC, P * BC), "sel")
            snb = load(d_snb, (128, 4), "snb")
            Wt = {}
            for l in range(5):
                for ki, (ks, kz) in enumerate(_kt(FIN[l])):
                    Wt[(l, ki)] = load(d_W[l][ks:ks + kz, :], (kz, FOUT[l]),
                                       f"W{l}_{ki}", F16)
            bv = (None if skip_bv else
                  [load(d_bv[l], (2, FOUT[l]), f"bv{l}", F16)
                   for l in range(5)])
            if not STREAM_WD1:
                Wd1s = load(d_Wd1s, (128, NSLOT * 4 * 512), "wd1s", F16)
            Wd2t = [load(d_Wd2[s:s + z, :], (z, 256), f"wd2_{s}", F16)
                    for s, z in _kt(512)]
            Wd3t = [load(d_Wd3[s:s + z, :], (z, 128), f"wd3_{s}", F16)
                    for s, z in _kt(256)]
            Wfct = load(d_Wfc, (128, 4), "wfc", F16)
            if skip_bd:
                bdr = [None, None, None, load(d_bdr[3], (1, 4), "bdr3")]
            else:
                bdr = [load(d_bdr[i], d_bdr[i].shape, f"bdr{i}")
                       for i in range(4)]
            ldg = ldb = None
            if not skip_ld:
                ldg = [load(d_ldg[i], d_ldg[i].shape, f"ldg{i}")
                       for i in range(3)]
                ldb = [load(d_ldb[i], d_ldb[i].shape, f"ldb{i}")
                       for i in range(3)]
            g5c = load(d_g5, (128, 4), "g5c")
            b5c = load(d_b5, (128, 4), "b5c")
            lngc = load(d_lng, (128, 4), "lngc")
            lnbc = load(d_lnb, (128, 4), "lnbc")

            ones_col = cp.tile([128, 1], F16, tag="ones_col")
            nc.vector.memset(ones_col[:], 1.0)
            ones_row = cp.tile([1, 128], F32, tag="ones_row")
            nc.vector.memset(ones_row[:], 1.0)
            ones_cb = cp.tile([128, 1], BF16, tag="ones_cb")
            nc.vector.memset(ones_cb[:], 1.0)

            yT = [cp.tile([128, BC * P], F16, tag="yT0", name="yT0"),
                  cp.tile([72, BC * P], F16, tag="yT1", name="yT1")]
            nc.sync.dma_start(out=yT[0][:], in_=d_y0T[0:128, :])
            nc.sync.dma_start(out=yT[1][:], in_=d_y0T[128:200, :])
            y_nat, U_nat, dl_nat, Atb_nat, ay = [
                [cp.tile([RG[g], N], F32, tag=f"{nm}{g}", name=f"{nm}{g}")
                 for g in range(4)]
                for nm in ("y", "U", "dl", "Atb", "ay")]
            yh = [cp.tile([RG[g], N], F16, tag=f"yh{g}", name=f"yh{g}")
                  for g in range(4)]
            for g in range(4):
                for tl, src in ((y_nat, d_y0n), (U_nat, d_U0n),
                                (dl_nat, d_d0n)):
                    nc.sync.dma_start(
                        out=tl[g][:], in_=src[RG0[g]:RG0[g] + RG[g], :])
            xnb = [cp.tile([z, BC * P], F16, tag=f"xnb{i}", name=f"xnb{i}")
                   for i, (s, z) in enumerate(KT400)]
            hT = [cp.tile([128, BC * P], F16, tag=f"h{i}", name=f"h{i}")
                  for i in range(4)]          # also holds enc^T after LN
            AtA = [cp.tile([128, P * N], F16, tag="ata0", name="ata0"),
                   cp.tile([72, P * N], F16, tag="ata1", name="ata1")]

            dram = es.enter_context(
                tc.tile_pool(name="ccdram", bufs=1, space="DRAM"))
            dram_nat = dram.tile([P * BC, N], F32, tag="dram_nat")
            dram_atb = dram.tile([P * BC, N], F32, tag="dram_atb")
            if not STREAM_WD1:
                dram_enc = dram.tile([8 * NSLOT * BC, 512], F16,
                                     tag="dram_enc")
                out_enc = dram.tile([8 * NSLOT * BC, 512], F16,
                                    tag="out_enc")
                in_z = dram.tile([8 * BC, 512], F32, tag="in_z")
                out_z = dram.tile([BC, 512], F32, tag="out_z")

            # ---------------- setup: AtA = A0^T A0, Atb = A0^T b ----------
            with tc.tile_pool(name="setup", bufs=1) as sp, \
                 tc.tile_pool(name="ps_setup", bufs=2, space="PSUM") as pps:
                bT = sp.tile([M, BC * P], F32, tag="bTs")
                nc.sync.dma_start(out=bT[:], in_=d_bT)
                # pad-slot zeroing for the enc exchange
                if not STREAM_WD1:
                    z8 = sp.tile([48, 512], F16, tag="z8")
                    nc.vector.memset(z8[:], 0.0)
                    pad_dmas = [nc.sync.dma_start(
                        out=dram_enc[400:448, :], in_=z8[:])]
                HP = P * N // 2
                for half in range(2):
                    A0 = sp.tile([M, HP], F32, tag="A0", name="A0")
                    nc.sync.dma_start(out=A0[:],
                                      in_=d_A0[:, half * HP:(half + 1) * HP])
                    A0h = sp.tile([M, HP], F16, tag="A0hh", name="A0hh")
                    nc.sync.dma_start(out=A0h[:],
                                      in_=d_A0h[:, half * HP:(half + 1) * HP])
                    for p in range(half * (P // 2), (half + 1) * (P // 2)):
                        a0p = A0[:, N * p - half * HP:N * p - half * HP + N]
                        a0ph = A0h[:, N * p - half * HP:N * p - half * HP + N]
                        for i, (s, z) in enumerate(KT200):
                            ps = pps.tile([z, N], F32, tag="sat", name="sat")
                            nc.tensor.matmul(ps[:], lhsT=a0ph[:, s:s + z],
                                             rhs=a0ph, start=True, stop=True)
                            nc.vector.tensor_copy(AtA[i][:, N * p:N * p + N],
                                                  ps[:])
                        # Atb for node p -> [8,200] psum tile -> group
                        # stage [8, NG*200] at free offsets -> DRAM bounce
                        # (reordered AP) -> natural group tile. Only DMA may
                        # address partition offsets that aren't 32-aligned.
                        g = _grp(p)
                        if p == PG0[g]:
                            stb = sp.tile([BC, NG[g] * N], F32, tag=f"stb{g}",
                                          name=f"stb{g}")
                        pab = pps.tile([BC, N], F32, tag="pab", name="pab",
                                       bufs=3)
                        nc.tensor.matmul(pab[:], lhsT=bT[:, 8 * p:8 * p + 8],
                                         rhs=a0p, start=True, stop=True)
                        pl = p - PG0[g]
                        nc.vector.tensor_copy(
                            stb[:, N * pl:N * pl + N], pab[:])
                        if p == PG0[g] + NG[g] - 1:
                            w = nc.sync.dma_start(
                                out=dram_atb[RG0[g]:RG0[g] + RG[g],
                                             :].rearrange(
                                    "(a s) f -> s a f", s=8),
                                in_=stb[:])
                            r = nc.sync.dma_start(
                                out=Atb_nat[g][:],
                                in_=dram_atb[RG0[g]:RG0[g] + RG[g], :])
                            add_dep_helper(r.ins, w.ins,
                                           reason="atb bounce RAW")
                for g in range(4):
                    for j, (js, jz) in enumerate(KT200):
                        pt = pps.tile([128, 128], F32, tag="satr",
                                      name="satr")
                        nc.tensor.transpose(pt[:jz, :RG[g]],
                                            Atb_nat[g][:, js:js + jz],
                                            ident[:RG[g], :RG[g]])
                        nc.vector.tensor_copy(
                            xnb[2 + j][:, RG0[g]:RG0[g] + RG[g]],
                            pt[:jz, :RG[g]])

            if STREAM_WD1:
                ring = es.enter_context(
                    tc.tile_pool(name="ring", bufs=RING_BUFS))

            cc_prev = {'aa': None, 'encP': None, 'rs': None, 'z1r': None}
            nat_prev = [None, None, None, None]
            # ---------------- K ADMM steps --------------------------------
            for k in range(K):
                sw = tc.alloc_tile_pool(name=f"sw{k}", bufs=1)
                # -- AtAy natural: quads of nodes per psum tile, J=200 --
                with tc.tile_pool(name=f"ps_aty{k}", bufs=4,
                                  space="PSUM") as pp:
                    for p in range(P):
                        g = _grp(p)
                        if p == PG0[g]:
                            sta = sw.tile([BC, NG[g] * N], F32,
                                          tag=f"stay{g}", name=f"stay{g}")
                        pa = pp.tile([BC, N], F32, tag="pa", name="pa")
                        for j, (js, jz) in enumerate(KT200):
                            nc.tensor.matmul(
                                pa[:],
                                lhsT=yT[j][:, 8 * p:8 * p + 8],
                                rhs=AtA[j][:, N * p:N * p + N],
                                start=(j == 0), stop=(j == 1))
                        pl = p - PG0[g]
                        nc.vector.tensor_copy(
                            sta[:, N * pl:N * pl + N], pa[:])
                        if p == PG0[g] + NG[g] - 1:
                            w = nc.sync.dma_start(
                                out=dram_nat[RG0[g]:RG0[g] + RG[g],
                                             :].rearrange(
                                    "(a s) f -> s a f", s=8),
                                in_=sta[:])
                            if nat_prev[g] is not None:
                                add_dep_helper(w.ins, nat_prev[g].ins,
                                               reason="nat bounce WAR")
                            r = nc.sync.dma_start(
                                out=ay[g][:],
                                in_=dram_nat[RG0[g]:RG0[g] + RG[g], :])
                            add_dep_helper(r.ins, w.ins,
                                           reason="nat bounce RAW")
                            nat_prev[g] = r
                # -- g = AtAy - Atb (natural); xnb[0:2] = AtAy^T (fp16) --
                g_t = [sw.tile([RG[g], N], F32, tag=f"g{g}", name=f"g{g}")
                       for g in range(4)]
                with tc.tile_pool(name=f"ps_atr{k}", bufs=2,
                                  space="PSUM") as pp:
                    for g in range(4):
                        nc.vector.tensor_sub(g_t[g][:], ay[g][:],
                                             Atb_nat[g][:])
                        for j, (js, jz) in enumerate(KT200):
                            pt = pp.tile([128, 128], F32, tag="atr",
                                         name="atr")
                            nc.tensor.transpose(pt[:jz, :RG[g]],
                                                ay[g][:, js:js + jz],
                                                ident[:RG[g], :RG[g]])
                            nc.vector.tensor_copy(
                                xnb[j][:, RG0[g]:RG0[g] + RG[g]],
                                pt[:jz, :RG[g]])

                # -- GCN: 5 layers; natural F per group, transposed agg --
                cur = xnb
                with tc.tile_pool(name=f"ps_gcn{k}", bufs=2,
                                  space="PSUM") as pp, \
                     tc.tile_pool(name=f"gcnw{k}", bufs=2) as gw:
                    for l in range(5):
                        fo = FOUT[l]
                        nxt = (hT if l == 4 else
                               [gw.tile([128, BC * P], F16, tag=f"xt{i}",
                                        name=f"xt{i}")
                                for i in range((fo + 127) // 128)])
                        Fsb = []
                        for g in range(4):
                            psF = pp.tile([RG[g], 512], F32, tag="F",
                                          name="F")
                            kts = _kt(FIN[l])
                            for ki, (ks, kz) in enumerate(kts):
                                nc.tensor.matmul(
                                    psF[:, :fo],
                                    lhsT=cur[ki][:, RG0[g]:RG0[g] + RG[g]],
                                    rhs=Wt[(l, ki)][:],
                                    start=(ki == 0), stop=(ki == len(kts) - 1))
                            fs = gw.tile([RG[g], 512], F16, tag=f"Fsb{g}",
                                         name=f"Fsb{g}")
                            nc.scalar.copy(fs[:, :fo], psF[:, :fo])
                            Fsb.append(fs)
                        for mi, (ms, mz) in enumerate(_kt(fo)):
                            psZ = pp.tile([128, BC * P], F32, tag="Z",
                                          name="Z")
                            for g in range(4):
                                nc.tensor.matmul(
                                    psZ[:mz, :],
                                    lhsT=Fsb[g][:, ms:ms + mz],
                                    rhs=GhB[g][:],
                                    start=(g == 0),
                                    stop=(g == 3 and skip_bv))
                            if not skip_bv:
                                nc.tensor.matmul(
                                    psZ[:mz, :], lhsT=bv[l][:, ms:ms + mz],
                                    rhs=GteN[:], start=False, stop=True)
                            nc.scalar.activation(nxt[mi][:mz, :], psZ[:mz, :],
                                                 AF.Lrelu, alpha=0.01)
                        cur = nxt

                # -- bn5 + LayerNorm over features (transposed) --
                with tc.tile_pool(name=f"ps_ln{k}", bufs=2, space="PSUM") as pp, \
                     tc.tile_pool(name=f"lnw{k}", bufs=1) as lw:
                    for i in range(4):
                        nc.vector.tensor_scalar(
                            hT[i][:], hT[i][:], g5c[:, i:i + 1],
                            b5c[:, i:i + 1], ALU.mult, ALU.add)
                    psmu = pp.tile([1, BC * P], F32, tag="mu", name="psmu")
                    for i in range(4):
                        nc.tensor.matmul(psmu[:], lhsT=ones_col[:],
                                         rhs=hT[i][:], start=(i == 0),
                                         stop=(i == 3))
                    mu_r = lw.tile([1, BC * P], F32, tag="mu_r")
                    nc.scalar.mul(mu_r[:], psmu[:], 1.0 / 512.0)
                    pssq = pp.tile([1, BC * P], F32, tag="sq", name="pssq")
                    for i in range(4):
                        hsq = lw.tile([128, BC * P], BF16, tag="hsq",
                                      name="hsq", bufs=2)
                        nc.vector.tensor_mul(hsq[:], hT[i][:], hT[i][:])
                        nc.tensor.matmul(pssq[:], lhsT=ones_cb[:], rhs=hsq[:],
                                         start=(i == 0), stop=(i == 3))
                    var_r = lw.tile([1, BC * P], F32, tag="var_r")
                    nc.scalar.mul(var_r[:], pssq[:], 1.0 / 512.0)
                    t_r = lw.tile([1, BC * P], F32, tag="t_r")
                    nc.vector.tensor_mul(t_r[:], mu_r[:], mu_r[:])
                    nc.vector.tensor_sub(var_r[:], var_r[:], t_r[:])
                    nc.vector.tensor_scalar_add(var_r[:], var_r[:], LN_EPS)
                    rs_r = _newton_rsqrt(nc, lw, var_r[:], [1, BC * P], "lnr")
                    bco = {}
                    for nm, row in (("mu", mu_r), ("rs", rs_r)):
                        psb = pp.tile([128, BC * P], F32, tag="bc" + nm,
                                      name="psb")
                        nc.tensor.matmul(psb[:], lhsT=ones_row[:], rhs=row[:],
                                         start=True, stop=True)
                        sb = lw.tile([128, BC * P], F16, tag="bcs" + nm,
                                     name="bcs")
                        nc.vector.tensor_copy(sb[:], psb[:])
                        bco[nm] = sb
                    for i in range(4):
                        nc.vector.tensor_sub(hT[i][:], hT[i][:], bco["mu"][:])
                        nc.vector.tensor_mul(hT[i][:], hT[i][:], bco["rs"][:])
                        nc.vector.tensor_scalar(
                            hT[i][:], hT[i][:], lngc[:, i:i + 1],
                            lnbc[:, i:i + 1], ALU.mult, ALU.add)

                # -- dense hyp-net --
                scal, nal = [], []
                with tc.tile_pool(name=f"ps_d{k}", bufs=1, space="PSUM") as pz, \
                     tc.tile_pool(name=f"ps_dt{k}", bufs=2, space="PSUM") as pt, \
                     tc.tile_pool(name=f"dw{k}", bufs=1) as dw:
                    if STREAM_WD1:
                        psz1 = pz.tile([BC, 512], F32, tag="z1", name="psz1")
                        for t in range(P):
                            rg = ring.tile([128, 4 * 512], F16, tag="wd1",
                                           name="wd1")
                            src = d_Wd1[512 * t:512 * t + 512, :].rearrange(
                                "(c r) n -> r c n", r=128)
                            nc.sync.dma_start(
                                out=rg[:].rearrange("r (c n) -> r c n", c=4),
                                in_=src)
                            for c in range(4):
                                nc.tensor.matmul(
                                    psz1[:], lhsT=hT[c][:, 8 * t:8 * t + 8],
                                    rhs=rg[:, 512 * c:512 * c + 512],
                                    start=(t == 0 and c == 0),
                                    stop=(t == P - 1 and c == 3 and skip_bd))
                        if not skip_bd:
                            nc.tensor.matmul(psz1[:], lhsT=ones_row[:, :BC],
                                             rhs=bdr[0][:], start=False,
                                             stop=True)
                        z1_src, z1_dim = psz1[:], 512
                    else:
                        # natural enc per group via PE transpose, DMA to the
                        # exchange buffer (1KB runs), AllToAll, partial z1
                        # for all 64 samples, ReduceScatter back.
                        enc_dmas = pad_dmas if k == 0 else []
                        for g in range(4):
                            encN = dw.tile([RG[g], 512], F16, tag=f"eN{g}",
                                           name=f"eN{g}")
                            for c in range(4):
                                pe = pt.tile([128, 128], F16, tag="pe",
                                             name="pe", bufs=2)
                                nc.tensor.transpose(
                                    pe[:RG[g], :],
                                    hT[c][:, RG0[g]:RG0[g] + RG[g]],
                                    identh[:128, :128])
                                nc.scalar.copy(
                                    encN[:, 128 * c:128 * c + 128],
                                    pe[:RG[g], :])
                            dd = nc.sync.dma_start(
                                out=dram_enc[RG0[g]:RG0[g] + RG[g], :],
                                in_=encN[:])
                            if cc_prev['aa'] is not None:
                                add_dep_helper(dd.ins, cc_prev['aa'].ins,
                                               reason="dram_enc WAR")
                            enc_dmas.append(dd)
                        aa = nc.gpsimd.collective_compute(
                            "AllToAll", ALU.bypass,
                            replica_groups=[list(range(NC_CORES))],
                            ins=[dram_enc.opt()], outs=[out_enc.opt()])
                        for dd in enc_dmas:
                            add_dep_helper(aa.ins, dd.ins,
                                           reason="enc ready before AllToAll")
                        if cc_prev['encP'] is not None:
                            for pe_d in cc_prev['encP']:
                                add_dep_helper(aa.ins, pe_d.ins,
                                               reason="out_enc WAR")
                        encP = dw.tile([8 * BC, NSLOT * 512], F16, tag="encP",
                                       name="encP")
                        eps = []
                        for i in range(8):
                            dd = nc.sync.dma_start(
                                out=encP[BC * i:BC * i + BC, :],
                                in_=out_enc[56 * i:56 * i + 56, :].rearrange(
                                    "(slot s) f -> s slot f", s=8))
                            add_dep_helper(dd.ins, aa.ins,
                                           reason="AllToAll before encP read")
                            eps.append(dd)
                        cc_prev['aa'], cc_prev['encP'] = aa, eps
                        psz1 = pz.tile([8 * BC, 512], F32, tag="z1",
                                       name="psz1")
                        for u in range(NSLOT * 4):
                            eT = dw.tile([128, 8 * BC], F16, tag="eT",
                                         name="eT", bufs=4)
                            pe = pt.tile([128, 128], F16, tag="pe",
                                         name="peT", bufs=2)
                            nc.tensor.transpose(
                                pe[:, :8 * BC],
                                encP[:, 128 * u:128 * u + 128],
                                identh[:8 * BC, :8 * BC])
                            nc.vector.tensor_copy(eT[:], pe[:, :8 * BC])
                            nc.tensor.matmul(
                                psz1[:], lhsT=eT[:],
                                rhs=Wd1s[:, 512 * u:512 * u + 512],
                                start=(u == 0), stop=(u == NSLOT * 4 - 1))
                        zp = dw.tile([8 * BC, 512], F32, tag="zp", name="zp")
                        nc.vector.tensor_copy(zp[:], psz1[:])
                        dz = nc.sync.dma_start(out=in_z[:], in_=zp[:])
                        if cc_prev['rs'] is not None:
                            add_dep_helper(dz.ins, cc_prev['rs'].ins,
                                           reason="in_z WAR")
                        rs = nc.gpsimd.collective_compute(
                            "ReduceScatter", ALU.add,
                            replica_groups=[list(range(NC_CORES))],
                            ins=[in_z.opt()], outs=[out_z.opt()])
                        add_dep_helper(rs.ins, dz.ins,
                                       reason="in_z ready before RS")
                        if cc_prev['z1r'] is not None:
                            add_dep_helper(rs.ins, cc_prev['z1r'].ins,
                                           reason="out_z WAR")
                        z1r = dw.tile([BC, 512], F32, tag="z1r", name="z1r")
                        dd = nc.sync.dma_start(out=z1r[:], in_=out_z[:])
                        add_dep_helper(dd.ins, rs.ins,
                                       reason="RS before z1r read")
                        cc_prev['rs'], cc_prev['z1r'] = rs, dd
                        if not skip_bd:
                            nc.vector.tensor_add(
                                z1r[:], z1r[:],
                                bdr[0][:].to_broadcast((BC, 512)))
                        z1_src, z1_dim = z1r[:], 512

                    def dense_ln_lrelu(ps_ap, fdim, li):
                        zr = dw.tile([BC, 512], F32, tag="d_zr", name="zr")
                        nc.vector.tensor_copy(zr[:, :fdim], ps_ap)
                        mu = dw.tile([BC, 1], F32, tag="d_mu", name="mu")
                        nc.vector.tensor_reduce(mu[:], zr[:, :fdim],
                                                mybir.AxisListType.X, ALU.add)
                        nc.vector.tensor_scalar_mul(mu[:], mu[:], 1.0 / fdim)
                        sq = dw.tile([BC, 512], F32, tag="d_sq", name="sq")
                        nc.vector.tensor_mul(sq[:, :fdim], zr[:, :fdim],
                                             zr[:, :fdim])
                        vr = dw.tile([BC, 1], F32, tag="d_vr", name="vr")
                        nc.vector.tensor_reduce(vr[:], sq[:, :fdim],
                                                mybir.AxisListType.X, ALU.add)
                        nc.vector.tensor_scalar_mul(vr[:], vr[:], 1.0 / fdim)
                        mm = dw.tile([BC, 1], F32, tag="d_mm", name="mm")
                        nc.vector.tensor_mul(mm[:], mu[:], mu[:])
                        nc.vector.tensor_sub(vr[:], vr[:], mm[:])
                        nc.vector.tensor_scalar_add(vr[:], vr[:], LN_EPS)
                        rs = _newton_rsqrt(nc, dw, vr[:], [BC, 1], "dn")
                        z = dw.tile([BC, 512], F16, tag="d_z", name="z")
                        nc.vector.tensor_scalar(z[:, :fdim], zr[:, :fdim],
                                                mu[:], rs[:],
                                                ALU.subtract, ALU.mult)
                        if not skip_ld:
                            nc.vector.tensor_mul(z[:, :fdim], z[:, :fdim],
                                                 ldg[li][:, :fdim])
                            nc.vector.tensor_add(z[:, :fdim], z[:, :fdim],
                                                 ldb[li][:, :fdim])
                        nc.vector.scalar_tensor_tensor(
                            z[:, :fdim], z[:, :fdim], 0.01, z[:, :fdim],
                            ALU.mult, ALU.max)
                        return z

                    def transpose_cols(z_sb, fdim):
                        outs = []
                        for ci, (cs, cz) in enumerate(_kt(fdim)):
                            pty = pt.tile([128, BC], F16, tag="zt",
                                          name="pty")
                            nc.tensor.transpose(pty[:cz, :],
                                                z_sb[:, cs:cs + cz],
                                                identh[:BC, :BC])
                            zz = dw.tile([128, BC], F16, tag="d_zt", bufs=4,
                                         name=f"zz{ci}")
                            nc.vector.tensor_copy(zz[:cz, :], pty[:cz, :])
                            outs.append((zz, cz))
                        return outs

                    z1 = dense_ln_lrelu(z1_src, z1_dim, 0)
                    z1t = transpose_cols(z1, 512)
                    psz2 = pz.tile([BC, 256], F32, tag="zs", name="psz2")
                    for ci, (zz, cz) in enumerate(z1t):
                        nc.tensor.matmul(psz2[:], lhsT=zz[:cz, :],
                                         rhs=Wd2t[ci][:], start=(ci == 0),
                                         stop=(ci == 3 and skip_bd))
                    if not skip_bd:
                        nc.tensor.matmul(psz2[:], lhsT=ones_row[:, :BC],
                                         rhs=bdr[1][:], start=False, stop=True)
                    z2 = dense_ln_lrelu(psz2[:], 256, 1)
                    z2t = transpose_cols(z2, 256)
                    psz3 = pz.tile([BC, 128], F32, tag="zs", name="psz3")
                    for ci, (zz, cz) in enumerate(z2t):
                        nc.tensor.matmul(psz3[:], lhsT=zz[:cz, :],
                                         rhs=Wd3t[ci][:], start=(ci == 0),
                                         stop=(ci == 1 and skip_bd))
                    if not skip_bd:
                        nc.tensor.matmul(psz3[:], lhsT=ones_row[:, :BC],
                                         rhs=bdr[2][:], start=False, stop=True)
                    z3 = dense_ln_lrelu(psz3[:], 128, 2)
                    z3t = transpose_cols(z3, 128)
                    psfc = pz.tile([BC, 4], F32, tag="zs", name="psfc")
                    nc.tensor.matmul(psfc[:], lhsT=z3t[0][0][:128, :],
                                     rhs=Wfct[:], start=True, stop=False)
                    nc.tensor.matmul(psfc[:], lhsT=ones_row[:, :BC],
                                     rhs=bdr[3][:], start=False, stop=True)
                    hyp = dw.tile([BC, 4], F32, tag="hyp", name="hyp")
                    nc.scalar.activation(hyp[:], psfc[:], AF.Sigmoid)
                    nc.vector.tensor_scalar(hyp[:], hyp[:], 0.9999, 1e-4,
                                            ALU.min, ALU.max)
                    for g in range(4):
                        pss = pt.tile([RG[g], 4], F32, tag="scal", name="pss")
                        nc.tensor.matmul(pss[:],
                                         lhsT=sel[:, RG0[g]:RG0[g] + RG[g]],
                                         rhs=hyp[:], start=True, stop=True)
                        sc = sw.tile([RG[g], 4], F32, tag=f"sc{g}",
                                     name=f"sc{g}")
                        nc.vector.tensor_copy(sc[:], pss[:])
                        na = sw.tile([RG[g], 1], F32, tag=f"na{g}",
                                     name=f"na{g}")
                        nc.vector.tensor_scalar_mul(na[:], sc[:, 0:1], -1.0)
                        scal.append(sc)
                        nal.append(na)

                # -- ADMM update (natural layout) --
                with tc.tile_pool(name=f"ps_ad{k}", bufs=2, space="PSUM") as pp, \
                     tc.tile_pool(name=f"adw{k}", bufs=2) as aw:
                    for g in range(4):
                        gg = g_t[g]
                        sg = aw.tile([RG[g], N], F32, tag="sg", name="sg")
                        nc.scalar.activation(sg[:], y_nat[g][:], AF.Sign)
                        nc.vector.scalar_tensor_tensor(
                            gg[:], sg[:], scal[g][:, 1:2], gg[:],
                            ALU.mult, ALU.add)
                        nc.vector.scalar_tensor_tensor(
                            gg[:], U_nat[g][:], snb[:RG[g], g:g + 1], gg[:],
                            ALU.mult, ALU.add)
                        nc.vector.scalar_tensor_tensor(
                            gg[:], dl_nat[g][:], scal[g][:, 2:3], gg[:],
                            ALU.mult, ALU.add)
                        nc.vector.tensor_scalar(gg[:], gg[:], 10.0, -10.0,
                                                ALU.min, ALU.max)
                        nc.vector.scalar_tensor_tensor(
                            y_nat[g][:], gg[:], nal[g][:], y_nat[g][:],
                            ALU.mult, ALU.add)
                        nc.vector.tensor_scalar(y_nat[g][:], y_nat[g][:],
                                                100.0, -100.0,
                                                ALU.min, ALU.max)
                        nc.scalar.copy(yh[g][:], y_nat[g][:])
                        if k < K - 1:
                            for j, (js, jz) in enumerate(KT200):
                                pty = pp.tile([128, 128], F16, tag="ytr",
                                              name="pty2")
                                nc.tensor.transpose(
                                    pty[:jz, :RG[g]], yh[g][:, js:js + jz],
                                    identh[:RG[g], :RG[g]])
                                nc.vector.tensor_copy(
                                    yT[j][:, RG0[g]:RG0[g] + RG[g]],
                                    pty[:jz, :RG[g]])
                        nc.sync.dma_start(
                            out=d_out[k, RG0[g]:RG0[g] + RG[g], :],
                            in_=y_nat[g][:])
                    for g in range(4):
                        psd = pp.tile([RG[g], N], F32, tag="dl", name="psd")
                        for gp in range(4):
                            nc.tensor.matmul(
                                psd[:], lhsT=Lh[gp][:, RG0[g]:RG0[g] + RG[g]],
                                rhs=yh[gp][:], start=(gp == 0), stop=(gp == 3))
                        nc.vector.tensor_scalar(dl_nat[g][:], psd[:],
                                                20.0, -20.0, ALU.min, ALU.max)
                        nc.vector.scalar_tensor_tensor(
                            U_nat[g][:], dl_nat[g][:], scal[g][:, 3:4],
                            U_nat[g][:], ALU.mult, ALU.add)
                        nc.vector.tensor_scalar(U_nat[g][:], U_nat[g][:],
                                                100.0, -100.0,
                                                ALU.min, ALU.max)
                sw.release()
    nc.compile()
    return nc


_NC_CACHE = {}


def _host_inputs(inputs):
    """Build all per-core DRAM arrays (numpy). Returns (in_maps, flags).
    Natural row index r = p*8 + b (node-major)."""
    f32 = np.float32
    f16 = np.float16
    BN_SCALE = f32(1.0) / np.sqrt(f32(1.0) + f32(1e-5))
    b_in = np.ascontiguousarray(np.asarray(inputs['b'], f32)[..., 0])
    A0 = np.ascontiguousarray(np.asarray(inputs['A'], f32)[0])
    edge = np.asarray(inputs['edge_index'])
    y0 = np.ascontiguousarray(np.asarray(inputs['y0'], f32)[..., 0])
    U0 = np.ascontiguousarray(np.asarray(inputs['U0'], f32)[..., 0])
    d0 = np.ascontiguousarray(np.asarray(inputs['delta0'], f32)[..., 0])

    Ws = [np.asarray(inputs['W%d' % i], f32) for i in range(1, 6)]
    bs = [np.asarray(inputs['b%d' % i], f32) for i in range(1, 6)]
    gs = [np.asarray(inputs['g%d' % i], f32) * BN_SCALE for i in range(1, 6)]
    bes = [np.asarray(inputs['be%d' % i], f32) for i in range(1, 6)]
    Wf = [Ws[0]] + [gs[l - 1][:, None] * Ws[l] for l in range(1, 5)]
    vs = [np.zeros(FOUT[0], f32)] + [(bes[l - 1] @ Ws[l]).astype(f32)
                                     for l in range(1, 5)]
    lds = [(np.asarray(inputs['ld%dg' % i], f32),
            np.asarray(inputs['ld%db' % i], f32)) for i in (1, 2, 3)]
    bds = [np.asarray(inputs['bd%d' % i], f32) for i in (1, 2, 3)]

    flags = dict(
        skip_bv=bool(all(np.all(bs[l] == 0) and np.all(vs[l] == 0)
                         for l in range(5))),
        skip_ld=bool(all(np.all(g == 1) and np.all(bb == 0)
                         for g, bb in lds)),
        skip_bd=bool(all(np.all(bd == 0) for bd in bds)),
    )

    A0l = A0.transpose(1, 0, 2).reshape(M, P * N)
    Wd1 = np.asarray(inputs['Wd1'], f32).astype(f16)
    shared = {'A0l': np.ascontiguousarray(A0l),
              'A0h': np.ascontiguousarray(A0l).astype(f16),
              'ident': np.eye(128, dtype=f32),
              'identh': np.eye(128, dtype=f16),
              'Wd2': np.asarray(inputs['Wd2'], f32).astype(f16),
              'Wd3': np.asarray(inputs['Wd3'], f32).astype(f16),
              'Wfc': np.asarray(inputs['Wfc'], f32).astype(f16),
              'bd1r': bds[0][None, :], 'bd2r': bds[1][None, :],
              'bd3r': bds[2][None, :],
              'bfcr': np.asarray(inputs['bfc'], f32)[None, :],
              'gs5c': np.ascontiguousarray(gs[4].reshape(4, 128).T),
              'be5c': np.ascontiguousarray(bes[4].reshape(4, 128).T),
              'lngc': np.ascontiguousarray(
                  np.asarray(inputs['ln_g'], f32).reshape(4, 128).T),
              'lnbc': np.ascontiguousarray(
                  np.asarray(inputs['ln_b'], f32).reshape(4, 128).T)}
    if STREAM_WD1:
        shared['Wd1'] = Wd1
    for l in range(5):
        shared[f'W{l+1}f'] = np.ascontiguousarray(Wf[l]).astype(f16)
        shared[f'bv{l+1}'] = np.stack([bs[l], vs[l]]).astype(f16)
    for i, nm in ((0, 'ld1'), (1, 'ld2'), (2, 'ld3')):
        g, bb = lds[i]
        shared[nm + 'gb'] = np.broadcast_to(g, (BC, g.size)).copy()
        shared[nm + 'bb'] = np.broadcast_to(bb, (BC, bb.size)).copy()
    # sel[b, r] = 1 iff r % 8 == b
    selm = np.zeros((BC, P * BC), f32)
    rr = np.arange(P * BC)
    selm[rr % BC, rr] = 1.0
    shared['sel'] = selm

    def natrows(x):  # [BC, P, N] -> [P*BC, N] with row p*8+b
        return np.ascontiguousarray(
            x.transpose(1, 0, 2).reshape(P * BC, N))

    in_maps = []
    for c in range(NC_CORES):
        sl = slice(BC * c, BC * c + BC)
        d = dict(shared)
        d['bT'] = np.ascontiguousarray(
            b_in[sl].transpose(2, 1, 0).reshape(M, P * BC))
        d['y0T'] = np.ascontiguousarray(
            y0[sl].transpose(2, 1, 0).reshape(N, P * BC)).astype(f16)
        d['y0n'] = natrows(y0[sl])
        d['U0n'] = natrows(U0[sl])
        d['d0n'] = natrows(d0[sl])
        GhBm = np.zeros((P * BC, P * BC), f16)
        GteNm = np.zeros((2, P * BC), f16)
        GteNm[0] = 1.0
        Lhm = np.zeros((P * BC, P * BC), f16)
        snbm = np.zeros((128, 4), f32)
        for bl in range(BC):
            bg = BC * c + bl
            s, dd = edge[bg, 0], edge[bg, 1]
            cnt = np.zeros((P, P), np.int64)
            np.add.at(cnt, (dd, s), 1)
            deg = (cnt.sum(1) + 1).astype(f32)
            nb = cnt.sum(0).astype(f32)
            G = (cnt.astype(f32)
                 / np.sqrt(deg[:, None] * deg[None, :]).astype(f32))
            G[np.arange(P), np.arange(P)] += (f32(1.0) / deg)
            L = 2.0 * (np.diag(nb) - cnt.astype(f32))
            qq, pp_ = np.meshgrid(np.arange(P), np.arange(P), indexing='ij')
            GhBm[qq * 8 + bl, pp_ * 8 + bl] = G.T.astype(f16)   # [q,p]=G[p,q]
            Lhm[qq * 8 + bl, pp_ * 8 + bl] = L.T.astype(f16)
            GteNm[1, np.arange(P) * 8 + bl] = G.sum(1).astype(f16)
            for g in range(4):
                rows = (np.arange(PG0[g], PG0[g] + NG[g]) - PG0[g]) * 8 + bl
                snbm[rows, g] = nb[PG0[g]:PG0[g] + NG[g]]
        d['GhB'] = GhBm
        d['GteN'] = GteNm
        d['Lh'] = Lhm
        d['snb'] = snbm
        if not STREAM_WD1:
            # shard: core c owns nodes t in [7c, 7c+7) (t>=50 are zero pad)
            W4 = Wd1.reshape(P, 4, 128, 512)
            sh = np.zeros((128, NSLOT * 4 * 512), f16)
            for i, t in enumerate(range(7 * c, 7 * c + 7)):
                if t >= P:
                    break
                for cc4 in range(4):
                    u = 4 * i + cc4
                    sh[:, 512 * u:512 * u + 512] = W4[t, cc4]
            d['Wd1s'] = sh
        in_maps.append(d)
    return in_maps, flags


def kernel(**inputs):
    in_maps, flags = _host_inputs(inputs)
    key = tuple(sorted(flags.items()))
    if key not in _NC_CACHE:
        _NC_CACHE[key] = build_nc(**flags)
    nc = _NC_CACHE[key]
    res = bass_utils.run_bass_kernel_spmd(nc, in_maps,
                                          core_ids=list(range(NC_CORES)))
    out = np.empty((K, B, P, N, 1), np.float32)
    for c in range(NC_CORES):
        ys = res.results[c]['Ys'].reshape(K, P, BC, N).transpose(0, 2, 1, 3)
        out[:, BC * c:BC * c + BC] = ys[..., None]
    return out
